# revision 1
# baseline (speedup 1.0000x reference)
"""Trainium2 Bass kernel for nn_Encoder (GCN layer + MLP/BatchNorm), 8 NeuronCores.

Strategy (per core, SPMD over 8 cores):
  Phase 1 (replicated): h = bf16(x @ W_gcn) written as a row-major DRAM table
    [Q*SEG, 256].  x arrives host-transposed (xT [512, Q*SEG] f32, zero-padded)
    so the stationary operand needs no on-device transpose; the f32->bf16 cast
    rides the SWDGE DMA.
  Phase 2 (sharded by destination row): edges are host-bucketed by
    (dest window of 128 rows, source quarter of the table).  For each bucket a
    gpsimd dma_gather pulls the source rows (512B bf16 each, int16 indices
    relative to the quarter) into SBUF; a one-instruction DVE tensor_scalar
    builds S^T[j,r] = val_j * (rowrel_j == r) from a constant iota tile; the
    segment-sum becomes PE matmuls accumulating into a PSUM window.  The
    window is PE-transposed so downstream work is column-major (h_aggT).
  Phase 3: z1 = W1-matmuls (W1 stationary), BatchNorm stats are reduced
    locally and AllReduce'd across the 8 cores (payload [128,4] f32), then the
    window is recomputed and Relu+affine applied in one ACT op; same for W2 /
    BN2, final affine written f32 to outT [2,128,RPC].

Host side does only index/layout work: degree-balanced node->window
assignment (LPT), edge bucketing/sorting, padding, and the output unpermute.
"""

import heapq
import numpy as np
import ml_dtypes

import concourse.bacc as bacc
from concourse import bass, mybir
from concourse.bass_utils import run_bass_kernel_spmd
from concourse.library_config import mlp

BF16 = ml_dtypes.bfloat16
F32 = mybir.dt.float32
BF = mybir.dt.bfloat16
AF = mybir.ActivationFunctionType
OP = mybir.AluOpType


class Cfg:
    def __init__(self, N=100000, E=3200000, SEG=25088, WPC=98, CQ=10, PH=3):
        self.N, self.E, self.SEG, self.WPC, self.CQ = N, E, SEG, WPC, CQ
        self.PH = PH
        self.CORES = 8
        self.Q = 4
        self.IN_C, self.HID, self.OUT_C = 512, 256, 256
        self.EPS = 1e-5
        self.TABROWS = self.Q * SEG          # h-table rows (>= N, %128 == 0)
        assert self.TABROWS >= N and self.TABROWS % 128 == 0
        assert SEG <= 32767 and SEG % 512 == 0  # quarter tensors batch-aligned
        self.RG = self.TABROWS // 128        # phase-1 row groups
        self.SL = (self.TABROWS + 511) // 512  # phase-1 slabs (last may be short)
        self.HB = (self.RG + 3) // 4         # phase-1 h-store batches of 4 rgs
        self.RPC = WPC * 128                 # rows per core (padded)
        self.ROWS_REAL = N // self.CORES     # real rows per core
        assert self.ROWS_REAL <= self.RPC
        self.NCH = (self.RPC + 511) // 512   # phase-3 row chunks
        self.GSLOT = 128 * CQ                # slots per (window, quarter)
        self.NG = WPC * self.Q               # gather groups per core
        self.GCT = self.NG * CQ              # total chunk columns per core
        # rings
        self.R_XT = 3
        self.R_IDX = min(12, WPC)
        self.R_RV = min(16, WPC)


def _ap(t, off, pattern):
    return bass.AP(t, off, pattern)


def build(c: Cfg):
    nc = bacc.Bacc("TRN2", debug=False)
    CQ, Q, WPC, SEG, HID = c.CQ, c.Q, c.WPC, c.SEG, c.HID

    xT = nc.declare_dram_parameter("xT", [c.IN_C, c.TABROWS], F32, isOutput=False)
    wgcn = nc.declare_dram_parameter("wgcn", [c.IN_C, HID], F32, isOutput=False)
    w1 = nc.declare_dram_parameter("w1", [HID, HID], F32, isOutput=False)
    w2 = nc.declare_dram_parameter("w2", [HID, c.OUT_C], F32, isOutput=False)
    bgcn2 = nc.declare_dram_parameter("bgcn2", [128, 2], F32, isOutput=False)
    g1v = nc.declare_dram_parameter("g1v", [128, 2], F32, isOutput=False)
    be1v = nc.declare_dram_parameter("be1v", [128, 2], F32, isOutput=False)
    g2v = nc.declare_dram_parameter("g2v", [128, 2], F32, isOutput=False)
    be2v = nc.declare_dram_parameter("be2v", [128, 2], F32, isOutput=False)
    iota_in = nc.declare_dram_parameter("iota", [128, 128], BF, isOutput=False)
    ident_in = nc.declare_dram_parameter("ident", [128, 128], BF, isOutput=False)
    idxw = nc.declare_dram_parameter("idxw", [128, WPC * 32 * CQ], mybir.dt.int16, isOutput=False)
    rrval = nc.declare_dram_parameter("rrval", [128, WPC * 8 * CQ], F32, isOutput=False)
    gcnt = nc.declare_dram_parameter("gcnt", [128, c.NG], mybir.dt.int32, isOutput=False)
    outT = nc.declare_dram_parameter("outT", [2, 128, c.RPC], F32, isOutput=True)

    htabs = [nc.dram_tensor(f"htab{i}", [SEG, HID], BF) for i in range(4)]
    cc1i = nc.dram_tensor("cc1i", [128, 4], F32)
    cc1o = nc.dram_tensor("cc1o", [128, 4], F32, addr_space="Shared")
    cc2i = nc.dram_tensor("cc2i", [128, 4], F32)
    cc2o = nc.dram_tensor("cc2o", [128, 4], F32, addr_space="Shared")

    from contextlib import ExitStack
    st_ctx = ExitStack()
    T = lambda name, shape, dt: st_ctx.enter_context(nc.sbuf_tensor(name, shape, dt))
    P = lambda name, shape, dt=F32: st_ctx.enter_context(nc.psum_tensor(name, shape, dt))
    S = lambda name: st_ctx.enter_context(nc.semaphore(name))

    with st_ctx:
        xts = T("xts", [128, c.R_XT, 4, 512], BF)
        wg = T("wg", [128, 4, HID], BF)
        w1s = T("w1s", [128, 2, 2, 128], BF)
        w2s = T("w2s", [128, 2, 2, 128], BF)
        ht = T("ht", [128, 2, 4, HID], BF)
        ev1 = T("ev1", [128, 2, HID], BF)
        gr = T("gr", [128, 4, CQ, HID], BF)
        ss = T("ss", [128, 3, CQ, 128], BF)
        ixs = T("ixs", [128, c.R_IDX, 32 * CQ], mybir.dt.int16)
        rvs = T("rvs", [128, c.R_RV, 8 * CQ], F32)
        cnt = T("cnt", [128, c.NG], mybir.dt.int32)
        io_sb = T("io_sb", [128, 128], BF)
        id_sb = T("id_sb", [128, 128], BF)
        hat = T("hat", [128, 2, c.RPC], BF)
        h1 = T("h1", [128, 2, c.RPC], BF)
        bg = T("bg", [128, 2], F32)
        g1s = T("g1s", [128, 2], F32)
        be1s = T("be1s", [128, 2], F32)
        g2s = T("g2s", [128, 2], F32)
        be2s = T("be2s", [128, 2], F32)
        stt = T("stt", [128, 2, 2, c.NCH], F32)
        ccp = T("ccp", [128, 4], F32)
        gst = T("gst", [128, 8], F32)
        kdt = T("kdt", [128, 16], F32)
        kd1 = T("kd1", [128, 4], F32)
        kd2 = T("kd2", [128, 4], F32)
        ot = T("ot", [128, 2, 512], F32)

        pa = [P("pa0", [128, HID]), P("pa1", [128, HID])]
        pt = [P("pt0", [128, 2, 128], BF), P("pt1", [128, 2, 128], BF)]
        p3 = [P(f"p3{i}", [128, 512]) for i in range(4)]

        s_pre = S("s_pre"); s_ms = S("s_ms")
        s_slab = [S(f"s_slab{i}") for i in range(c.R_XT)]
        s_p1ps = S("s_p1ps"); s_p1ev = S("s_p1ev")
        s_p1st = [S(f"s_p1st{i}") for i in range(2)]
        s_idx = [S(f"s_idx{i}") for i in range(c.R_IDX)]
        s_rv = [S(f"s_rv{i}") for i in range(c.R_RV)]
        s_g = [S(f"s_g{i}") for i in range(4)]
        s_s = S("s_s")
        s_pg = S("s_pg"); s_e1 = S("s_e1"); s_pt = S("s_pt"); s_e2 = S("s_e2")
        s_3ps = S("s_3ps"); s_3c = S("s_3c"); s_sq = S("s_sq"); s_h1 = S("s_h1"); s_oev = S("s_oev")
        s_ost = [S(f"s_ost{i}") for i in range(2)]
        s_stf = S("s_stf"); s_cio = S("s_cio"); s_cc = S("s_cc")
        s_kd = S("s_kd")

        N_PRE = 11 * 16
        # rows in each phase-1 slab / rgs per slab
        slab_rows = [min(512, c.TABROWS - 512 * sl) for sl in range(c.SL)]
        rg_end = np.cumsum([r // 128 for r in slab_rows]).tolist()  # rgs done after slab sl
        batch_rgs = [min(4, c.RG - 4 * b) for b in range(c.HB)]
        # phase-3 chunk rows
        rows_t = [min(512, c.RPC - 512 * t) for t in range(c.NCH)]
        rreal_t = [max(0, min(rows_t[t], c.ROWS_REAL - 512 * t)) for t in range(c.NCH)]
        assert all(r > 0 for r in rreal_t)
        NT = 2 * c.NCH          # tiles per phase-3 pass
        qbs = 16 * c.HB         # s_p1st target when table fully stored

        with nc.Block() as block:

            @block.gpsimd
            def _(g: bass.BassGpSimd):
                g.load_library(mlp)
                # ---- preloads (11 DMAs) ----
                g.dma_start(wg[:, :, :], _ap(wgcn, 0, [[HID, 128], [128 * HID, 4], [1, HID]])).then_inc(s_pre, 16)
                g.dma_start(w1s[:, :, :, :], _ap(w1, 0, [[HID, 128], [128 * HID, 2], [128, 2], [1, 128]])).then_inc(s_pre, 16)
                g.dma_start(w2s[:, :, :, :], _ap(w2, 0, [[HID, 128], [128 * HID, 2], [128, 2], [1, 128]])).then_inc(s_pre, 16)
                g.dma_start(io_sb[:, :], iota_in[:, :]).then_inc(s_pre, 16)
                g.dma_start(id_sb[:, :], ident_in[:, :]).then_inc(s_pre, 16)
                g.dma_start(bg[:, :], bgcn2[:, :]).then_inc(s_pre, 16)
                g.dma_start(g1s[:, :], g1v[:, :]).then_inc(s_pre, 16)
                g.dma_start(be1s[:, :], be1v[:, :]).then_inc(s_pre, 16)
                g.dma_start(g2s[:, :], g2v[:, :]).then_inc(s_pre, 16)
                g.dma_start(be2s[:, :], be2v[:, :]).then_inc(s_pre, 16)
                g.dma_start(cnt[:, :], gcnt[:, :]).then_inc(s_pre, 16)
                # ---- phase 1: xT slabs (f32 -> bf16 cast in DMA) + h-table stores ----
                BPQ = SEG // 512  # h-store batches per quarter

                def h_store(b):
                    nt = batch_rgs[b]
                    qb, lb = divmod(b, BPQ)
                    g.wait_ge(s_p1ev, min(4 * b + nt, c.RG))
                    g.dma_start(
                        _ap(htabs[qb], lb * 512 * HID, [[HID, 128], [128 * HID, nt], [1, HID]]),
                        ht[:, b % 2, 0:nt, :],
                    ).then_inc(s_p1st[b % 2], 16)
                for sl in range(c.SL):
                    if sl >= c.R_XT:
                        g.wait_ge(s_p1ps, rg_end[sl - c.R_XT])
                    rows = slab_rows[sl]
                    g.dma_start(
                        xts[:, sl % c.R_XT, :, 0:rows],
                        _ap(xT, 512 * sl, [[c.TABROWS, 128], [128 * c.TABROWS, 4], [1, rows]]),
                    ).then_inc(s_slab[sl % c.R_XT], 16)
                    if sl >= 2:
                        h_store(sl - 2)
                for b in range(max(0, c.SL - 2), c.HB):
                    h_store(b)
                if c.PH < 2:
                    g.wait_ge(s_p1st[0], 16 * ((c.HB + 1) // 2))
                    g.wait_ge(s_p1st[1], 16 * (c.HB // 2))
                    return
                # ---- phase 2: gathers ----
                g.wait_ge(s_p1st[0], 16 * ((c.HB + 1) // 2))
                g.wait_ge(s_p1st[1], 16 * (c.HB // 2))
                g.wait_ge(s_ms, 4)
                with g.register("cntreg") as creg:
                    for w in range(WPC):
                        g.wait_ge(s_idx[w % c.R_IDX], 16 * (w // c.R_IDX + 1))
                        for q in range(Q):
                            gi = Q * w + q
                            if gi >= 4:
                                g.wait_ge(s_pg, gi - 3)
                            g.reg_load(creg, _ap(cnt, gi, [[c.NG, 1], [1, 1]]))
                            g.dma_gather(
                                gr[:, gi % 4, :, :],
                                htabs[q][:, :],
                                ixs[:, w % c.R_IDX, q * 8 * CQ:(q + 1) * 8 * CQ],
                                c.GSLOT, creg, HID, single_packet=False,
                            ).then_inc(s_g[gi % 4], 16)
                if c.PH < 3:
                    return
                # ---- phase 3: stats AllReduce x2 ----
                g.wait_ge(s_stf, 1)
                g.dma_start(cc1i[:, :], ccp[:, :]).then_inc(s_cio, 16)
                g.wait_ge(s_cio, 16)
                g.collective_compute("AllReduce", OP.add, replica_groups=[list(range(c.CORES))],
                                     ins=[cc1i.ap().opt()], outs=[cc1o.ap().opt()]).then_inc(s_cc, 1)
                g.wait_ge(s_cc, 1)
                g.dma_start(gst[:, 0:4], cc1o[:, :]).then_inc(s_cio, 16)
                g.wait_ge(s_stf, 2)
                g.dma_start(cc2i[:, :], ccp[:, :]).then_inc(s_cio, 16)
                g.wait_ge(s_cio, 48)
                g.collective_compute("AllReduce", OP.add, replica_groups=[list(range(c.CORES))],
                                     ins=[cc2i.ap().opt()], outs=[cc2o.ap().opt()]).then_inc(s_cc, 1)
                g.wait_ge(s_cc, 2)
                g.dma_start(gst[:, 4:8], cc2o[:, :]).then_inc(s_cio, 16)
                g.wait_ge(s_cio, 64)

            @block.sync
            def _(sp):
                if c.PH < 2:
                    return
                # phase 2 idx + rrval window streams
                for w in range(WPC):
                    if w >= c.R_IDX:
                        wp = w - c.R_IDX
                        for q in range(Q):
                            sp.wait_ge(s_g[q], 16 * (wp + 1))
                    sp.dma_start(ixs[:, w % c.R_IDX, :], idxw[:, w * 32 * CQ:(w + 1) * 32 * CQ]).then_inc(s_idx[w % c.R_IDX], 16)
                    if w >= c.R_RV:
                        sp.wait_ge(s_s, CQ * Q * (w - c.R_RV + 1))
                    sp.dma_start(rvs[:, w % c.R_RV, :], rrval[:, w * 8 * CQ:(w + 1) * 8 * CQ]).then_inc(s_rv[w % c.R_RV], 16)
                if c.PH < 3:
                    for w in range(max(0, WPC - c.R_IDX), WPC):
                        sp.wait_ge(s_idx[w % c.R_IDX], 16 * (w // c.R_IDX + 1))
                        sp.wait_ge(s_rv[w % c.R_RV], 16 * (w // c.R_RV + 1))
                    return
                # phase 3 out stores
                for i in range(NT):
                    hf, t = divmod(i, c.NCH)
                    sp.wait_ge(s_oev, i + 1)
                    sp.dma_start(
                        _ap(outT, hf * 128 * c.RPC + t * 512, [[c.RPC, 128], [1, rows_t[t]]]),
                        ot[:, i % 2, 0:rows_t[t]],
                    ).then_inc(s_ost[i % 2], 16)
                sp.wait_ge(s_ost[0], 16 * ((NT + 1) // 2))
                sp.wait_ge(s_ost[1], 16 * (NT // 2))

            @block.tensor
            def _(pe: bass.BassTensorEngine):
                pe.wait_ge(s_pre, N_PRE)
                # ---- phase 1 matmuls ----
                for sl in range(c.SL):
                    pe.wait_ge(s_slab[sl % c.R_XT], 16 * (sl // c.R_XT + 1))
                    for j in range(slab_rows[sl] // 128):
                        rg = 4 * sl + j
                        if rg >= 2:
                            pe.wait_ge(s_p1ev, rg - 1)
                        for kc in range(4):
                            mm = pe.matmul(
                                pa[rg % 2][:, :],
                                xts[:, sl % c.R_XT, kc, 128 * j:128 * (j + 1)],
                                wg[:, kc, :],
                                start=(kc == 0), stop=(kc == 3),
                            )
                            if kc == 3:
                                mm.then_inc(s_p1ps, 1)
                if c.PH < 2:
                    return
                # ---- phase 2 scatter matmuls + window transposes ----
                for w in range(WPC):
                    if w >= 2:
                        pe.wait_ge(s_e1, w - 1)
                    for q in range(Q):
                        gi = Q * w + q
                        pe.wait_ge(s_g[gi % 4], 16 * (gi // 4 + 1))
                        pe.wait_ge(s_s, CQ * (gi + 1))
                        for ch in range(CQ):
                            mm = pe.matmul(
                                pa[w % 2][:, :],
                                ss[:, gi % 3, ch, :],
                                gr[:, gi % 4, ch, :],
                                start=(q == 0 and ch == 0), stop=(q == Q - 1 and ch == CQ - 1),
                            )
                            if ch == CQ - 1:
                                mm.then_inc(s_pg, 1)
                    if w >= 1:
                        v = w - 1
                        pe.wait_ge(s_e1, v + 1)
                        if v >= 2:
                            pe.wait_ge(s_e2, 2 * v - 2)
                        for i in range(2):
                            pe.matmul(pt[v % 2][:, i, :], ev1[:, v % 2, 128 * i:128 * (i + 1)],
                                      id_sb[:, :], is_transpose=True, start=True, stop=True).then_inc(s_pt, 1)
                v = WPC - 1
                pe.wait_ge(s_e1, v + 1)
                pe.wait_ge(s_e2, max(0, 2 * v - 2))
                for i in range(2):
                    pe.matmul(pt[v % 2][:, i, :], ev1[:, v % 2, 128 * i:128 * (i + 1)],
                              id_sb[:, :], is_transpose=True, start=True, stop=True).then_inc(s_pt, 1)
                if c.PH < 3:
                    return
                # ---- phase 3: 4 passes x (2 halves x NCH chunks) ----
                pe.wait_ge(s_e2, 2 * WPC)
                for i in range(4 * NT):
                    p, j = divmod(i, NT)
                    hf, t = divmod(j, c.NCH)
                    if i >= 4:
                        pp, jj = divmod(i - 4, NT)
                        if pp == 0:
                            pe.wait_ge(s_sq, jj + 1)
                        elif pp == 1:
                            pe.wait_ge(s_h1, jj + 1)
                        elif pp == 2:
                            pe.wait_ge(s_sq, NT + jj + 1)
                        else:
                            pe.wait_ge(s_oev, jj + 1)
                    if p == 2:
                        pe.wait_ge(s_h1, c.NCH + t + 1)
                    ws = w1s if p < 2 else w2s
                    src = hat if p < 2 else h1
                    rt = rows_t[t]
                    for ci in range(2):
                        mm = pe.matmul(
                            p3[i % 4][:, 0:rt],
                            ws[:, ci, hf, :],
                            src[:, ci, 512 * t:512 * t + rt],
                            start=(ci == 0), stop=(ci == 1),
                        )
                        if ci == 1:
                            mm.then_inc(s_3ps, 1)

            @block.vector
            def _(v: bass.BassVectorEngine):
                for sl4 in range(4):
                    v.memset(gr[:, sl4, :, :], 0).then_inc(s_ms, 1)
                v.wait_ge(s_pre, N_PRE)
                if c.PH < 2:
                    return
                # ---- phase 2 S-builds ----
                for w in range(WPC):
                    v.wait_ge(s_rv[w % c.R_RV], 16 * (w // c.R_RV + 1))
                    for q in range(Q):
                        gi = Q * w + q
                        if gi >= 3:
                            v.wait_ge(s_pg, gi - 2)
                        for ch in range(CQ):
                            v.tensor_scalar(
                                ss[:, gi % 3, ch, :], io_sb[:, :],
                                rvs[:, w % c.R_RV, 2 * (q * CQ + ch):2 * (q * CQ + ch) + 1],
                                rvs[:, w % c.R_RV, 2 * (q * CQ + ch) + 1:2 * (q * CQ + ch) + 2],
                                OP.is_equal, OP.mult,
                            ).then_inc(s_s, 1)
                if c.PH < 3:
                    return
                # ---- phase 3 ----
                for layer in range(2):
                    base = 0 if layer == 0 else 2 * NT
                    for j in range(NT):
                        hf, t = divmod(j, c.NCH)
                        v.wait_ge(s_3ps, base + j + 1)
                        rr = rreal_t[t]
                        psl = p3[(base + j) % 4]
                        v.tensor_reduce(stt[:, hf, 0, t:t + 1], psl[:, 0:rr],
                                        mybir.AxisListType.X, OP.add).then_inc(s_3c, 1)
                    v.wait_ge(s_sq, NT * (layer + 1))
                    v.drain()
                    v.tensor_reduce(ccp[:, 0:1], stt[:, 0, 0, :], mybir.AxisListType.X, OP.add)
                    v.tensor_reduce(ccp[:, 1:2], stt[:, 0, 1, :], mybir.AxisListType.X, OP.add)
                    v.tensor_reduce(ccp[:, 2:3], stt[:, 1, 0, :], mybir.AxisListType.X, OP.add)
                    v.tensor_reduce(ccp[:, 3:4], stt[:, 1, 1, :], mybir.AxisListType.X, OP.add)
                    v.drain().then_inc(s_stf, 1)
                    # finalize after AllReduce
                    v.wait_ge(s_cio, 32 + 32 * layer)
                    gof = 4 * layer
                    sums = _ap(gst, gof, [[8, 128], [2, 2]])
                    sqs = _ap(gst, gof + 1, [[8, 128], [2, 2]])
                    inv_n = 1.0 / c.N
                    v.tensor_scalar(kdt[:, 0:2], sums, inv_n, None, OP.mult)
                    v.tensor_scalar(kdt[:, 2:4], sqs, inv_n, None, OP.mult)
                    v.drain()
                    v.tensor_mul(kdt[:, 4:6], kdt[:, 0:2], kdt[:, 0:2])
                    v.drain()
                    v.tensor_sub(kdt[:, 6:8], kdt[:, 2:4], kdt[:, 4:6])
                    v.drain()
                    v.tensor_scalar(kdt[:, 6:8], kdt[:, 6:8], c.EPS, None, OP.add)
                    v.drain().then_inc(s_kd, 1)
                    v.wait_ge(s_kd, 2 + 3 * layer)
                    v.reciprocal(kdt[:, 10:12], kdt[:, 8:10])
                    v.drain()
                    kd = kd1 if layer == 0 else kd2
                    gv = g1s if layer == 0 else g2s
                    bev = be1s if layer == 0 else be2s
                    v.tensor_mul(kd[:, 0:2], gv[:, :], kdt[:, 10:12])
                    v.drain()
                    v.tensor_mul(kdt[:, 12:14], kdt[:, 0:2], kd[:, 0:2])
                    v.drain()
                    v.tensor_sub(kd[:, 2:4], bev[:, :], kdt[:, 12:14])
                    v.drain().then_inc(s_kd, 1)

            @block.scalar
            def _(a: bass.BassScalarEngine):
                a.wait_ge(s_pre, N_PRE)
                # ---- phase 1 psum evacuation (f32 -> bf16) ----
                for rg in range(c.RG):
                    a.wait_ge(s_p1ps, rg + 1)
                    b = rg // 4
                    if b >= 2 and rg % 4 == 0:
                        a.wait_ge(s_p1st[b % 2], 16 * ((b - 2) // 2 + 1))
                    a.activation(ht[:, b % 2, rg % 4, :], pa[rg % 2][:, :], AF.Identity).then_inc(s_p1ev, 1)
                if c.PH < 2:
                    return
                # ---- phase 2 evacuations ----
                for w in range(WPC):
                    a.wait_ge(s_pg, Q * (w + 1))
                    if w >= 2:
                        a.wait_ge(s_pt, 2 * w - 2)
                    a.activation(ev1[:, w % 2, :], pa[w % 2][:, :], AF.Identity).then_inc(s_e1, 1)
                    if w >= 1:
                        vv = w - 1
                        a.wait_ge(s_pt, 2 * (vv + 1))
                        for i in range(2):
                            a.activation(hat[:, i, 128 * vv:128 * (vv + 1)], pt[vv % 2][:, i, :],
                                         AF.Identity, bias=bg[:, i:i + 1]).then_inc(s_e2, 1)
                vv = WPC - 1
                a.wait_ge(s_pt, 2 * (vv + 1))
                for i in range(2):
                    a.activation(hat[:, i, 128 * vv:128 * (vv + 1)], pt[vv % 2][:, i, :],
                                 AF.Identity, bias=bg[:, i:i + 1]).then_inc(s_e2, 1)
                if c.PH < 3:
                    return
                # ---- phase 3 ----
                for layer in range(2):
                    sbase = 0 if layer == 0 else 2 * NT
                    for j in range(NT):
                        hf, t = divmod(j, c.NCH)
                        a.wait_ge(s_3ps, sbase + j + 1)
                        a.wait_ge(s_3c, NT * layer + j + 1)
                        rr = rreal_t[t]
                        psl = p3[(sbase + j) % 4]
                        a.activation(psl[:, 0:rr], psl[:, 0:rr], AF.Square,
                                     accum_out=stt[:, hf, 1, t:t + 1]).then_inc(s_sq, 1)
                    # sqrt step for k/d
                    a.wait_ge(s_kd, 1 + 3 * layer)
                    a.sqrt(kdt[:, 8:10], kdt[:, 6:8]).then_inc(s_kd, 1)
                    a.wait_ge(s_kd, 3 + 3 * layer)
                    kd = kd1 if layer == 0 else kd2
                    pbase = NT if layer == 0 else 3 * NT
                    for j in range(NT):
                        hf, t = divmod(j, c.NCH)
                        a.wait_ge(s_3ps, pbase + j + 1)
                        rt = rows_t[t]
                        psl = p3[(pbase + j) % 4]
                        if layer == 0:
                            a.activation(h1[:, hf, 512 * t:512 * t + rt], psl[:, 0:rt], AF.Relu,
                                         bias=kd[:, 2 + hf:3 + hf], scale=kd[:, hf:hf + 1]).then_inc(s_h1, 1)
                        else:
                            if j >= 2:
                                a.wait_ge(s_ost[j % 2], 16 * ((j - 2) // 2 + 1))
                            a.activation(ot[:, j % 2, 0:rt], psl[:, 0:rt], AF.Identity,
                                         bias=kd[:, 2 + hf:3 + hf], scale=kd[:, hf:hf + 1]).then_inc(s_oev, 1)

        nc.compile()
    return nc


# ---------------------------------------------------------------------------
# host-side preprocessing
# ---------------------------------------------------------------------------

def preprocess(x, edge_row, edge_col, edge_val, c: Cfg):
    N, E, WPC, Q, SEG = c.N, len(edge_row), c.WPC, c.Q, c.SEG
    deg = np.bincount(edge_row, minlength=N)
    order = np.argsort(-deg, kind="stable")
    rank = np.empty(N, np.int64)
    rank[order] = np.arange(N)
    core = (rank % c.CORES).astype(np.int32)

    win_of = np.empty(N, np.int32)
    slot_of = np.empty(N, np.int32)
    caps = np.full(WPC, 128, np.int32)
    tail = c.ROWS_REAL - 128 * (WPC - 1)
    caps[WPC - 1] = tail if tail > 0 else 128
    # capacity sanity: total capacity >= rows_real
    assert caps.sum() >= c.ROWS_REAL
    for k in range(c.CORES):
        nodes = order[k::c.CORES]
        heap = [(0, w) for w in range(WPC)]
        heapq.heapify(heap)
        fill = np.zeros(WPC, np.int32)
        for n in nodes:
            s, w = heapq.heappop(heap)
            win_of[n] = w
            slot_of[n] = fill[w]
            fill[w] += 1
            if fill[w] < caps[w]:
                heapq.heappush(heap, (s + int(deg[n]), w))
    ek = core[edge_row]
    ew = win_of[edge_row]
    er = slot_of[edge_row]
    eq = (edge_col // SEG).astype(np.int64)
    erel = (edge_col - eq * SEG).astype(np.int16)
    key = ((ek.astype(np.int64) * WPC + ew) * Q + eq)
    sidx = np.argsort(key, kind="stable")
    key_s = key[sidx]
    ngroups = c.CORES * WPC * Q
    counts = np.bincount(key_s, minlength=ngroups)
    cnt128 = np.maximum((counts + 127) // 128, 1)
    cq_needed = int(cnt128.max())
    if cq_needed > c.CQ:
        return None, cq_needed  # caller rebuilds with larger CQ
    GSLOT = c.GSLOT
    starts = np.zeros(ngroups, np.int64)
    starts[1:] = np.cumsum(counts)[:-1]
    pos = np.arange(E) - starts[key_s]
    gslot = key_s * GSLOT + pos
    TOT = ngroups * GSLOT
    idx_sl = np.full(TOT, -1, np.int16)
    rr_sl = np.zeros(TOT, np.float32)
    val_sl = np.zeros(TOT, np.float32)
    idx_sl[gslot] = erel[sidx]
    rr_sl[gslot] = er[sidx].astype(np.float32)
    val_sl[gslot] = np.asarray(edge_val, np.float32)[sidx]
    sig = np.arange(TOT, dtype=np.int64) % GSLOT
    gof = np.arange(TOT, dtype=np.int64) // GSLOT
    padmask = (sig >= counts[gof]) & (sig < cnt128[gof] * 128)
    idx_sl[padmask] = 0
    gcnt_all = (cnt128 * 128).astype(np.int32)

    xTp = np.zeros((c.IN_C, c.TABROWS), np.float32)
    xTp[:, :N] = np.asarray(x, np.float32).T

    per_core = []
    idx_c = idx_sl.reshape(c.CORES, WPC, Q * GSLOT)
    rr_c = rr_sl.reshape(c.CORES, -1, 128)
    val_c = val_sl.reshape(c.CORES, -1, 128)
    for k in range(c.CORES):
        a = idx_c[k].reshape(WPC, Q * GSLOT // 16, 16)
        w16 = np.transpose(a, (2, 0, 1)).reshape(16, WPC * Q * GSLOT // 16)
        idxw_k = np.ascontiguousarray(np.tile(w16, (8, 1)))
        rrT = rr_c[k].T  # [128, GCT]
        valT = val_c[k].T
        rrval_k = np.ascontiguousarray(np.stack([rrT, valT], axis=-1).reshape(128, -1))
        per_core.append(dict(idxw=idxw_k, rrval=rrval_k,
                             gcnt=np.ascontiguousarray(np.tile(gcnt_all.reshape(c.CORES, -1)[k:k + 1], (128, 1)))))
    meta = dict(core=core, win_of=win_of, slot_of=slot_of, xTp=xTp)
    return (per_core, meta), None


def make_in_maps(inputs, c: Cfg):
    res, cq_needed = preprocess(inputs["x"], np.asarray(inputs["edge_row"]),
                                np.asarray(inputs["edge_col"]), np.asarray(inputs["edge_val"]), c)
    if res is None:
        return None, cq_needed
    per_core, meta = res
    iota = np.broadcast_to(np.arange(128, dtype=np.float32), (128, 128)).astype(BF16)
    ident = np.eye(128, dtype=np.float32).astype(BF16)

    def v2(b):
        return np.ascontiguousarray(np.asarray(b, np.float32).reshape(2, 128).T)

    shared = dict(
        xT=meta["xTp"],
        wgcn=np.asarray(inputs["W_gcn"], np.float32),
        w1=np.asarray(inputs["W1"], np.float32),
        w2=np.asarray(inputs["W2"], np.float32),
        bgcn2=v2(inputs["b_gcn"]), g1v=v2(inputs["g1"]), be1v=v2(inputs["be1"]),
        g2v=v2(inputs["g2"]), be2v=v2(inputs["be2"]),
        iota=np.ascontiguousarray(iota), ident=np.ascontiguousarray(ident),
    )
    in_maps = [dict(shared, **pc) for pc in per_core]
    return (in_maps, meta), None


def unshard(results, meta, c: Cfg):
    core, win_of, slot_of = meta["core"], meta["win_of"], meta["slot_of"]
    out = np.empty((c.N, c.OUT_C), np.float32)
    rowpos = win_of.astype(np.int64) * 128 + slot_of
    for k in range(c.CORES):
        o = np.asarray(results[k]["outT"]).reshape(2, 128, c.RPC)
        o = np.transpose(o, (2, 0, 1)).reshape(c.RPC, c.OUT_C)
        nodes_k = np.flatnonzero(core == k)
        out[nodes_k] = o[rowpos[nodes_k]]
    return out


_NC_CACHE = {}


def get_nc(c: Cfg):
    key = (c.N, c.SEG, c.WPC, c.CQ, c.PH)
    if key not in _NC_CACHE:
        _NC_CACHE[key] = build(c)
    return _NC_CACHE[key]


def kernel(**inputs):
    c = Cfg()
    while True:
        res, cq_needed = make_in_maps(inputs, c)
        if res is not None:
            break
        c = Cfg(CQ=cq_needed)
    in_maps, meta = res
    nc = get_nc(c)
    r = run_bass_kernel_spmd(nc, in_maps, list(range(c.CORES)))
    return unshard(r.results, meta, c)



# revision 8
# speedup vs baseline: 4.8511x; 4.8511x over previous
"""Trainium2 Bass kernel for nn_Encoder (GCN layer + MLP/BatchNorm), 8 NeuronCores.

Strategy (per core, SPMD over 8 cores):
  Phase 1 (replicated): h = bf16(x @ W_gcn) written as a row-major DRAM table
    [Q*SEG, 256].  x arrives host-transposed (xT [512, Q*SEG] f32, zero-padded)
    so the stationary operand needs no on-device transpose; the f32->bf16 cast
    rides the SWDGE DMA.
  Phase 2 (sharded by destination row): edges are host-bucketed by
    (dest window of 128 rows, source quarter of the table).  For each bucket a
    gpsimd dma_gather pulls the source rows (512B bf16 each, int16 indices
    relative to the quarter) into SBUF; a one-instruction DVE tensor_scalar
    builds S^T[j,r] = val_j * (rowrel_j == r) from a constant iota tile; the
    segment-sum becomes PE matmuls accumulating into a PSUM window.  The
    window is PE-transposed so downstream work is column-major (h_aggT).
  Phase 3: z1 = W1-matmuls (W1 stationary), BatchNorm stats are reduced
    locally and AllReduce'd across the 8 cores (payload [128,4] f32), then the
    window is recomputed and Relu+affine applied in one ACT op; same for W2 /
    BN2, final affine written f32 to outT [2,128,RPC].

Host side does only index/layout work: degree-balanced node->window
assignment (LPT), edge bucketing/sorting, padding, and the output unpermute.
"""

import heapq
import numpy as np
import ml_dtypes

import concourse.bacc as bacc
from concourse import bass, mybir
from concourse.bass_utils import run_bass_kernel_spmd
from concourse.library_config import mlp

BF16 = ml_dtypes.bfloat16
F32 = mybir.dt.float32
BF = mybir.dt.bfloat16
AF = mybir.ActivationFunctionType
OP = mybir.AluOpType


class Cfg:
    def __init__(self, N=100000, E=3200000, SEG=25088, WPC=98, CQ=10, PH=3):
        self.N, self.E, self.SEG, self.WPC, self.CQ = N, E, SEG, WPC, CQ
        self.PH = PH
        self.CORES = 8
        self.Q = 4
        self.IN_C, self.HID, self.OUT_C = 512, 256, 256
        self.EPS = 1e-5
        self.TABROWS = self.Q * SEG          # h-table rows (>= N, %128 == 0)
        assert self.TABROWS >= N and self.TABROWS % 128 == 0
        assert SEG <= 32767 and SEG % 512 == 0  # quarter tensors batch-aligned
        self.RG = self.TABROWS // 128        # phase-1 row groups
        self.SL = (self.TABROWS + 511) // 512  # phase-1 slabs (last may be short)
        self.HB = (self.RG + 3) // 4         # phase-1 h-store batches of 4 rgs
        self.RPC = WPC * 128                 # rows per core (padded)
        self.ROWS_REAL = N // self.CORES     # real rows per core
        assert self.ROWS_REAL <= self.RPC
        self.NCH = (self.RPC + 511) // 512   # phase-3 row chunks
        self.GSLOT = 128 * CQ                # slots per (window, quarter)
        self.NG = WPC * self.Q               # gather groups per core
        self.GCT = self.NG * CQ              # total chunk columns per core
        # rings
        self.R_XT = 3
        self.R_IDX = min(12, WPC)
        self.R_RV = min(16, WPC)


def _ap(t, off, pattern):
    return bass.AP(t, off, pattern)


def build(c: Cfg):
    nc = bacc.Bacc("TRN2", debug=False)
    CQ, Q, WPC, SEG, HID = c.CQ, c.Q, c.WPC, c.SEG, c.HID

    xT = nc.declare_dram_parameter("xT", [c.IN_C, c.TABROWS], BF, isOutput=False)
    wgcn = nc.declare_dram_parameter("wgcn", [c.IN_C, HID], F32, isOutput=False)
    w1 = nc.declare_dram_parameter("w1", [HID, HID], F32, isOutput=False)
    w2 = nc.declare_dram_parameter("w2", [HID, c.OUT_C], F32, isOutput=False)
    bgcn2 = nc.declare_dram_parameter("bgcn2", [128, 2], F32, isOutput=False)
    g1v = nc.declare_dram_parameter("g1v", [128, 2], F32, isOutput=False)
    be1v = nc.declare_dram_parameter("be1v", [128, 2], F32, isOutput=False)
    g2v = nc.declare_dram_parameter("g2v", [128, 2], F32, isOutput=False)
    be2v = nc.declare_dram_parameter("be2v", [128, 2], F32, isOutput=False)
    iota_in = nc.declare_dram_parameter("iota", [128, 128], BF, isOutput=False)
    ident_in = nc.declare_dram_parameter("ident", [128, 128], BF, isOutput=False)
    idxw = nc.declare_dram_parameter("idxw", [128, WPC * 32 * CQ], mybir.dt.int16, isOutput=False)
    rrval = nc.declare_dram_parameter("rrval", [128, WPC * 8 * CQ], F32, isOutput=False)
    gcnt = nc.declare_dram_parameter("gcnt", [128, c.NG], mybir.dt.int32, isOutput=False)
    outT = nc.declare_dram_parameter("outT", [2, 128, c.RPC], F32, isOutput=True)

    htabs = [nc.dram_tensor(f"htab{i}", [SEG, HID], BF) for i in range(4)]
    cc1i = nc.dram_tensor("cc1i", [128, 4], F32)
    cc1o = nc.dram_tensor("cc1o", [128, 4], F32, addr_space="Shared")
    cc2i = nc.dram_tensor("cc2i", [128, 4], F32)
    cc2o = nc.dram_tensor("cc2o", [128, 4], F32, addr_space="Shared")

    from contextlib import ExitStack
    st_ctx = ExitStack()
    T = lambda name, shape, dt: st_ctx.enter_context(nc.sbuf_tensor(name, shape, dt))
    P = lambda name, shape, dt=F32: st_ctx.enter_context(nc.psum_tensor(name, shape, dt))
    S = lambda name: st_ctx.enter_context(nc.semaphore(name))

    with st_ctx:
        xts = T("xts", [128, c.R_XT, 4, 512], BF)
        wg = T("wg", [128, 4, HID], BF)
        w1s = T("w1s", [128, 2, 2, 128], BF)
        w2s = T("w2s", [128, 2, 2, 128], BF)
        ht = T("ht", [128, 2, 4, HID], BF)
        ev1 = T("ev1", [128, 2, HID], BF)
        gr = T("gr", [128, 4, CQ, HID], BF)
        ss = T("ss", [128, 3, CQ, 128], BF)
        ixs = T("ixs", [128, c.R_IDX, 32 * CQ], mybir.dt.int16)
        rvs = T("rvs", [128, c.R_RV, 8 * CQ], F32)
        cnt = T("cnt", [128, c.NG], mybir.dt.int32)
        io_sb = T("io_sb", [128, 128], BF)
        id_sb = T("id_sb", [128, 128], BF)
        hat = T("hat", [128, 2, c.RPC], BF)
        h1 = T("h1", [128, 2, c.RPC], BF)
        bg = T("bg", [128, 2], F32)
        g1s = T("g1s", [128, 2], F32)
        be1s = T("be1s", [128, 2], F32)
        g2s = T("g2s", [128, 2], F32)
        be2s = T("be2s", [128, 2], F32)
        stt = T("stt", [128, 2, 2, c.NCH], F32)
        ccp = T("ccp", [128, 4], F32)
        gst = T("gst", [128, 8], F32)
        kdt = T("kdt", [128, 16], F32)
        kd1 = T("kd1", [128, 4], F32)
        kd2 = T("kd2", [128, 4], F32)
        ot = T("ot", [128, 2, 512], F32)

        pa = [P("pa0", [128, HID]), P("pa1", [128, HID])]
        pt = [P("pt0", [128, 2, 128], BF), P("pt1", [128, 2, 128], BF)]
        p3 = [P(f"p3{i}", [128, 512]) for i in range(4)]

        s_pre = S("s_pre"); s_ms = S("s_ms")
        s_slab = [S(f"s_slab{i}") for i in range(c.R_XT)]
        s_p1ps = S("s_p1ps"); s_p1ev = S("s_p1ev")
        s_p1st = [S(f"s_p1st{i}") for i in range(2)]
        s_idx = [S(f"s_idx{i}") for i in range(c.R_IDX)]
        s_rv = [S(f"s_rv{i}") for i in range(c.R_RV)]
        s_g = [S(f"s_g{i}") for i in range(4)]
        s_s = S("s_s")
        s_pg = S("s_pg"); s_e1 = S("s_e1"); s_pt = S("s_pt"); s_e2 = S("s_e2")
        s_3ps = S("s_3ps"); s_3c = S("s_3c"); s_sq = S("s_sq"); s_h1 = S("s_h1"); s_oev = S("s_oev")
        s_ost = [S(f"s_ost{i}") for i in range(2)]
        s_stf = S("s_stf"); s_cio = S("s_cio"); s_cc = S("s_cc")
        s_kd = S("s_kd")

        N_PRE = 11 * 16
        # rows in each phase-1 slab / rgs per slab
        slab_rows = [min(512, c.TABROWS - 512 * sl) for sl in range(c.SL)]
        rg_end = np.cumsum([r // 128 for r in slab_rows]).tolist()  # rgs done after slab sl
        batch_rgs = [min(4, c.RG - 4 * b) for b in range(c.HB)]
        # phase-3 chunk rows
        rows_t = [min(512, c.RPC - 512 * t) for t in range(c.NCH)]
        rreal_t = [max(0, min(rows_t[t], c.ROWS_REAL - 512 * t)) for t in range(c.NCH)]
        assert all(r > 0 for r in rreal_t)
        NT = 2 * c.NCH          # tiles per phase-3 pass
        qbs = 16 * c.HB         # s_p1st target when table fully stored

        with nc.Block() as block:

            @block.gpsimd
            def _(g: bass.BassGpSimd):
                g.load_library(mlp)
                # ---- preloads (11 DMAs) ----
                g.dma_start(wg[:, :, :], _ap(wgcn, 0, [[HID, 128], [128 * HID, 4], [1, HID]])).then_inc(s_pre, 16)
                g.dma_start(w1s[:, :, :, :], _ap(w1, 0, [[HID, 128], [128 * HID, 2], [128, 2], [1, 128]])).then_inc(s_pre, 16)
                g.dma_start(w2s[:, :, :, :], _ap(w2, 0, [[HID, 128], [128 * HID, 2], [128, 2], [1, 128]])).then_inc(s_pre, 16)
                g.dma_start(io_sb[:, :], iota_in[:, :]).then_inc(s_pre, 16)
                g.dma_start(id_sb[:, :], ident_in[:, :]).then_inc(s_pre, 16)
                g.dma_start(bg[:, :], bgcn2[:, :]).then_inc(s_pre, 16)
                g.dma_start(g1s[:, :], g1v[:, :]).then_inc(s_pre, 16)
                g.dma_start(be1s[:, :], be1v[:, :]).then_inc(s_pre, 16)
                g.dma_start(g2s[:, :], g2v[:, :]).then_inc(s_pre, 16)
                g.dma_start(be2s[:, :], be2v[:, :]).then_inc(s_pre, 16)
                g.dma_start(cnt[:, :], gcnt[:, :]).then_inc(s_pre, 16)
                # phase-1 slab loads / h-stores moved to sync / scalar (HWDGE)
                # so the Q7 SWDGE is free for phase-2 gather descriptor gen
                if c.PH < 2:
                    g.wait_ge(s_p1st[0], 16 * ((c.HB + 1) // 2))
                    g.wait_ge(s_p1st[1], 16 * (c.HB // 2))
                    return
                # ---- phase 2: gathers ----
                g.wait_ge(s_p1st[0], 16 * ((c.HB + 1) // 2))
                g.wait_ge(s_p1st[1], 16 * (c.HB // 2))
                g.wait_ge(s_ms, 4)
                with g.register("cntreg") as creg:
                    for w in range(WPC):
                        g.wait_ge(s_idx[w % c.R_IDX], 16 * (w // c.R_IDX + 1))
                        for q in range(Q):
                            gi = Q * w + q
                            if gi >= 4:
                                g.wait_ge(s_pg, gi - 3)
                            g.reg_load(creg, _ap(cnt, gi, [[c.NG, 1], [1, 1]]))
                            g.dma_gather(
                                gr[:, gi % 4, :, :],
                                htabs[q][:, :],
                                ixs[:, w % c.R_IDX, q * 8 * CQ:(q + 1) * 8 * CQ],
                                c.GSLOT, creg, HID, single_packet=False,
                            ).then_inc(s_g[gi % 4], 16)
                if c.PH < 3:
                    return
                # ---- phase 3: stats AllReduce x2 ----
                g.wait_ge(s_stf, 1)
                g.dma_start(cc1i[:, :], ccp[:, :]).then_inc(s_cio, 16)
                g.wait_ge(s_cio, 16)
                g.collective_compute("AllReduce", OP.add, replica_groups=[list(range(c.CORES))],
                                     ins=[cc1i.ap().opt()], outs=[cc1o.ap().opt()]).then_inc(s_cc, 1)
                g.wait_ge(s_cc, 1)
                g.dma_start(gst[:, 0:4], cc1o[:, :]).then_inc(s_cio, 16)
                g.wait_ge(s_stf, 2)
                g.dma_start(cc2i[:, :], ccp[:, :]).then_inc(s_cio, 16)
                g.wait_ge(s_cio, 48)
                g.collective_compute("AllReduce", OP.add, replica_groups=[list(range(c.CORES))],
                                     ins=[cc2i.ap().opt()], outs=[cc2o.ap().opt()]).then_inc(s_cc, 1)
                g.wait_ge(s_cc, 2)
                g.dma_start(gst[:, 4:8], cc2o[:, :]).then_inc(s_cio, 16)
                g.wait_ge(s_cio, 64)

            @block.sync
            def _(sp):
                # ---- phase 1 xT slab loads (HWDGE) ----
                for sl in range(c.SL):
                    if sl >= c.R_XT:
                        sp.wait_ge(s_p1ps, rg_end[sl - c.R_XT])
                    rows = slab_rows[sl]
                    sp.dma_start(
                        xts[:, sl % c.R_XT, :, 0:rows],
                        _ap(xT, 512 * sl, [[c.TABROWS, 128], [128 * c.TABROWS, 4], [1, rows]]),
                    ).then_inc(s_slab[sl % c.R_XT], 16)
                if c.PH < 2:
                    return
                # phase 2 idx + rrval window streams
                for w in range(WPC):
                    if w >= c.R_IDX:
                        wp = w - c.R_IDX
                        for q in range(Q):
                            sp.wait_ge(s_g[q], 16 * (wp + 1))
                    sp.dma_start(ixs[:, w % c.R_IDX, :], idxw[:, w * 32 * CQ:(w + 1) * 32 * CQ]).then_inc(s_idx[w % c.R_IDX], 16)
                    if w >= c.R_RV:
                        sp.wait_ge(s_s, CQ * Q * (w - c.R_RV + 1))
                    sp.dma_start(rvs[:, w % c.R_RV, :], rrval[:, w * 8 * CQ:(w + 1) * 8 * CQ]).then_inc(s_rv[w % c.R_RV], 16)
                if c.PH < 3:
                    for w in range(max(0, WPC - c.R_IDX), WPC):
                        sp.wait_ge(s_idx[w % c.R_IDX], 16 * (w // c.R_IDX + 1))
                        sp.wait_ge(s_rv[w % c.R_RV], 16 * (w // c.R_RV + 1))
                    return
                # phase 3 out stores
                for i in range(NT):
                    hf, t = divmod(i, c.NCH)
                    sp.wait_ge(s_oev, i + 1)
                    sp.dma_start(
                        _ap(outT, hf * 128 * c.RPC + t * 512, [[c.RPC, 128], [1, rows_t[t]]]),
                        ot[:, i % 2, 0:rows_t[t]],
                    ).then_inc(s_ost[i % 2], 16)
                sp.wait_ge(s_ost[0], 16 * ((NT + 1) // 2))
                sp.wait_ge(s_ost[1], 16 * (NT // 2))

            @block.tensor
            def _(pe: bass.BassTensorEngine):
                pe.wait_ge(s_pre, N_PRE)
                # ---- phase 1 matmuls ----
                for sl in range(c.SL):
                    pe.wait_ge(s_slab[sl % c.R_XT], 16 * (sl // c.R_XT + 1))
                    for j in range(slab_rows[sl] // 128):
                        rg = 4 * sl + j
                        if rg >= 2:
                            pe.wait_ge(s_p1ev, rg - 1)
                        for kc in range(4):
                            mm = pe.matmul(
                                pa[rg % 2][:, :],
                                xts[:, sl % c.R_XT, kc, 128 * j:128 * (j + 1)],
                                wg[:, kc, :],
                                start=(kc == 0), stop=(kc == 3),
                            )
                            if kc == 3:
                                mm.then_inc(s_p1ps, 1)
                if c.PH < 2:
                    return
                # ---- phase 2 scatter matmuls + window transposes ----
                for w in range(WPC):
                    if w >= 2:
                        pe.wait_ge(s_e1, w - 1)
                    for q in range(Q):
                        gi = Q * w + q
                        pe.wait_ge(s_g[gi % 4], 16 * (gi // 4 + 1))
                        pe.wait_ge(s_s, CQ * (gi + 1))
                        for ch in range(CQ):
                            mm = pe.matmul(
                                pa[w % 2][:, :],
                                ss[:, gi % 3, ch, :],
                                gr[:, gi % 4, ch, :],
                                start=(q == 0 and ch == 0), stop=(q == Q - 1 and ch == CQ - 1),
                            )
                            if ch == CQ - 1:
                                mm.then_inc(s_pg, 1)
                    if w >= 1:
                        v = w - 1
                        pe.wait_ge(s_e1, v + 1)
                        if v >= 2:
                            pe.wait_ge(s_e2, 2 * v - 2)
                        for i in range(2):
                            pe.matmul(pt[v % 2][:, i, :], ev1[:, v % 2, 128 * i:128 * (i + 1)],
                                      id_sb[:, :], is_transpose=True, start=True, stop=True).then_inc(s_pt, 1)
                v = WPC - 1
                pe.wait_ge(s_e1, v + 1)
                pe.wait_ge(s_e2, max(0, 2 * v - 2))
                for i in range(2):
                    pe.matmul(pt[v % 2][:, i, :], ev1[:, v % 2, 128 * i:128 * (i + 1)],
                              id_sb[:, :], is_transpose=True, start=True, stop=True).then_inc(s_pt, 1)
                if c.PH < 3:
                    return
                # ---- phase 3: 4 passes x (2 halves x NCH chunks) ----
                pe.wait_ge(s_e2, 2 * WPC)
                for i in range(4 * NT):
                    p, j = divmod(i, NT)
                    hf, t = divmod(j, c.NCH)
                    if i >= 4:
                        pp, jj = divmod(i - 4, NT)
                        if pp == 0:
                            pe.wait_ge(s_sq, jj + 1)
                        elif pp == 1:
                            pe.wait_ge(s_h1, jj + 1)
                        elif pp == 2:
                            pe.wait_ge(s_sq, NT + jj + 1)
                        else:
                            pe.wait_ge(s_oev, jj + 1)
                    if p == 2:
                        pe.wait_ge(s_h1, c.NCH + t + 1)
                    ws = w1s if p < 2 else w2s
                    src = hat if p < 2 else h1
                    rt = rows_t[t]
                    for ci in range(2):
                        mm = pe.matmul(
                            p3[i % 4][:, 0:rt],
                            ws[:, ci, hf, :],
                            src[:, ci, 512 * t:512 * t + rt],
                            start=(ci == 0), stop=(ci == 1),
                        )
                        if ci == 1:
                            mm.then_inc(s_3ps, 1)

            @block.vector
            def _(v: bass.BassVectorEngine):
                for sl4 in range(4):
                    v.memset(gr[:, sl4, :, :], 0).then_inc(s_ms, 1)
                v.wait_ge(s_pre, N_PRE)
                if c.PH < 2:
                    return
                # ---- phase 2 S-builds ----
                for w in range(WPC):
                    v.wait_ge(s_rv[w % c.R_RV], 16 * (w // c.R_RV + 1))
                    for q in range(Q):
                        gi = Q * w + q
                        if gi >= 3:
                            v.wait_ge(s_pg, gi - 2)
                        for ch in range(CQ):
                            v.tensor_scalar(
                                ss[:, gi % 3, ch, :], io_sb[:, :],
                                rvs[:, w % c.R_RV, 2 * (q * CQ + ch):2 * (q * CQ + ch) + 1],
                                rvs[:, w % c.R_RV, 2 * (q * CQ + ch) + 1:2 * (q * CQ + ch) + 2],
                                OP.is_equal, OP.mult,
                            ).then_inc(s_s, 1)
                if c.PH < 3:
                    return
                # ---- phase 3 ----
                for layer in range(2):
                    base = 0 if layer == 0 else 2 * NT
                    for j in range(NT):
                        hf, t = divmod(j, c.NCH)
                        v.wait_ge(s_3ps, base + j + 1)
                        rr = rreal_t[t]
                        psl = p3[(base + j) % 4]
                        v.tensor_reduce(stt[:, hf, 0, t:t + 1], psl[:, 0:rr],
                                        mybir.AxisListType.X, OP.add).then_inc(s_3c, 1)
                    v.wait_ge(s_sq, NT * (layer + 1))
                    v.drain()
                    v.tensor_reduce(ccp[:, 0:1], stt[:, 0, 0, :], mybir.AxisListType.X, OP.add)
                    v.tensor_reduce(ccp[:, 1:2], stt[:, 0, 1, :], mybir.AxisListType.X, OP.add)
                    v.tensor_reduce(ccp[:, 2:3], stt[:, 1, 0, :], mybir.AxisListType.X, OP.add)
                    v.tensor_reduce(ccp[:, 3:4], stt[:, 1, 1, :], mybir.AxisListType.X, OP.add)
                    v.drain().then_inc(s_stf, 1)
                    # finalize after AllReduce
                    v.wait_ge(s_cio, 32 + 32 * layer)
                    gof = 4 * layer
                    sums = _ap(gst, gof, [[8, 128], [2, 2]])
                    sqs = _ap(gst, gof + 1, [[8, 128], [2, 2]])
                    inv_n = 1.0 / c.N
                    v.tensor_scalar(kdt[:, 0:2], sums, inv_n, None, OP.mult)
                    v.tensor_scalar(kdt[:, 2:4], sqs, inv_n, None, OP.mult)
                    v.drain()
                    v.tensor_mul(kdt[:, 4:6], kdt[:, 0:2], kdt[:, 0:2])
                    v.drain()
                    v.tensor_sub(kdt[:, 6:8], kdt[:, 2:4], kdt[:, 4:6])
                    v.drain()
                    v.tensor_scalar(kdt[:, 6:8], kdt[:, 6:8], c.EPS, None, OP.add)
                    v.drain().then_inc(s_kd, 1)
                    v.wait_ge(s_kd, 2 + 3 * layer)
                    v.reciprocal(kdt[:, 10:12], kdt[:, 8:10])
                    v.drain()
                    kd = kd1 if layer == 0 else kd2
                    gv = g1s if layer == 0 else g2s
                    bev = be1s if layer == 0 else be2s
                    v.tensor_mul(kd[:, 0:2], gv[:, :], kdt[:, 10:12])
                    v.drain()
                    v.tensor_mul(kdt[:, 12:14], kdt[:, 0:2], kd[:, 0:2])
                    v.drain()
                    v.tensor_sub(kd[:, 2:4], bev[:, :], kdt[:, 12:14])
                    v.drain().then_inc(s_kd, 1)

            @block.scalar
            def _(a: bass.BassScalarEngine):
                a.wait_ge(s_pre, N_PRE)
                # ---- phase 1 psum evacuation (f32 -> bf16) + h-store (HWDGE) ----
                BPQ = SEG // 512  # h-store batches per quarter
                for rg in range(c.RG):
                    a.wait_ge(s_p1ps, rg + 1)
                    b = rg // 4
                    if b >= 2 and rg % 4 == 0:
                        a.wait_ge(s_p1st[b % 2], 16 * ((b - 2) // 2 + 1))
                    a.activation(ht[:, b % 2, rg % 4, :], pa[rg % 2][:, :], AF.Identity).then_inc(s_p1ev, 1)
                    if rg == 4 * b + batch_rgs[b] - 1:
                        nt = batch_rgs[b]
                        qb, lb = divmod(b, BPQ)
                        a.dma_start(
                            _ap(htabs[qb], lb * 512 * HID, [[HID, 128], [128 * HID, nt], [1, HID]]),
                            ht[:, b % 2, 0:nt, :],
                        ).then_inc(s_p1st[b % 2], 16)
                if c.PH < 2:
                    return
                # ---- phase 2 evacuations ----
                for w in range(WPC):
                    a.wait_ge(s_pg, Q * (w + 1))
                    if w >= 2:
                        a.wait_ge(s_pt, 2 * w - 2)
                    a.activation(ev1[:, w % 2, :], pa[w % 2][:, :], AF.Identity).then_inc(s_e1, 1)
                    if w >= 1:
                        vv = w - 1
                        a.wait_ge(s_pt, 2 * (vv + 1))
                        for i in range(2):
                            a.activation(hat[:, i, 128 * vv:128 * (vv + 1)], pt[vv % 2][:, i, :],
                                         AF.Identity, bias=bg[:, i:i + 1]).then_inc(s_e2, 1)
                vv = WPC - 1
                a.wait_ge(s_pt, 2 * (vv + 1))
                for i in range(2):
                    a.activation(hat[:, i, 128 * vv:128 * (vv + 1)], pt[vv % 2][:, i, :],
                                 AF.Identity, bias=bg[:, i:i + 1]).then_inc(s_e2, 1)
                if c.PH < 3:
                    return
                # ---- phase 3 ----
                for layer in range(2):
                    sbase = 0 if layer == 0 else 2 * NT
                    for j in range(NT):
                        hf, t = divmod(j, c.NCH)
                        a.wait_ge(s_3ps, sbase + j + 1)
                        a.wait_ge(s_3c, NT * layer + j + 1)
                        rr = rreal_t[t]
                        psl = p3[(sbase + j) % 4]
                        a.activation(psl[:, 0:rr], psl[:, 0:rr], AF.Square,
                                     accum_out=stt[:, hf, 1, t:t + 1]).then_inc(s_sq, 1)
                    # sqrt step for k/d
                    a.wait_ge(s_kd, 1 + 3 * layer)
                    a.sqrt(kdt[:, 8:10], kdt[:, 6:8]).then_inc(s_kd, 1)
                    a.wait_ge(s_kd, 3 + 3 * layer)
                    kd = kd1 if layer == 0 else kd2
                    pbase = NT if layer == 0 else 3 * NT
                    for j in range(NT):
                        hf, t = divmod(j, c.NCH)
                        a.wait_ge(s_3ps, pbase + j + 1)
                        rt = rows_t[t]
                        psl = p3[(pbase + j) % 4]
                        if layer == 0:
                            a.activation(h1[:, hf, 512 * t:512 * t + rt], psl[:, 0:rt], AF.Relu,
                                         bias=kd[:, 2 + hf:3 + hf], scale=kd[:, hf:hf + 1]).then_inc(s_h1, 1)
                        else:
                            if j >= 2:
                                a.wait_ge(s_ost[j % 2], 16 * ((j - 2) // 2 + 1))
                            a.activation(ot[:, j % 2, 0:rt], psl[:, 0:rt], AF.Identity,
                                         bias=kd[:, 2 + hf:3 + hf], scale=kd[:, hf:hf + 1]).then_inc(s_oev, 1)

        nc.compile()
    return nc


# ---------------------------------------------------------------------------
# host-side preprocessing
# ---------------------------------------------------------------------------

def preprocess(x, edge_row, edge_col, edge_val, c: Cfg):
    N, E, WPC, Q, SEG = c.N, len(edge_row), c.WPC, c.Q, c.SEG
    deg = np.bincount(edge_row, minlength=N)
    order = np.argsort(-deg, kind="stable")
    rank = np.empty(N, np.int64)
    rank[order] = np.arange(N)
    core = (rank % c.CORES).astype(np.int32)

    win_of = np.empty(N, np.int32)
    slot_of = np.empty(N, np.int32)
    caps = np.full(WPC, 128, np.int32)
    tail = c.ROWS_REAL - 128 * (WPC - 1)
    caps[WPC - 1] = tail if tail > 0 else 128
    # capacity sanity: total capacity >= rows_real
    assert caps.sum() >= c.ROWS_REAL
    for k in range(c.CORES):
        nodes = order[k::c.CORES]
        heap = [(0, w) for w in range(WPC)]
        heapq.heapify(heap)
        fill = np.zeros(WPC, np.int32)
        for n in nodes:
            s, w = heapq.heappop(heap)
            win_of[n] = w
            slot_of[n] = fill[w]
            fill[w] += 1
            if fill[w] < caps[w]:
                heapq.heappush(heap, (s + int(deg[n]), w))
    ek = core[edge_row]
    ew = win_of[edge_row]
    er = slot_of[edge_row]
    eq = (edge_col // SEG).astype(np.int64)
    erel = (edge_col - eq * SEG).astype(np.int16)
    key = ((ek.astype(np.int64) * WPC + ew) * Q + eq)
    # secondary sort by source row: ascending gather addresses within a group
    sidx = np.lexsort((erel, key))
    key_s = key[sidx]
    ngroups = c.CORES * WPC * Q
    counts = np.bincount(key_s, minlength=ngroups)
    cnt128 = np.maximum((counts + 127) // 128, 1)
    cq_needed = int(cnt128.max())
    if cq_needed > c.CQ:
        return None, cq_needed  # caller rebuilds with larger CQ
    GSLOT = c.GSLOT
    starts = np.zeros(ngroups, np.int64)
    starts[1:] = np.cumsum(counts)[:-1]
    pos = np.arange(E) - starts[key_s]
    gslot = key_s * GSLOT + pos
    TOT = ngroups * GSLOT
    idx_sl = np.full(TOT, -1, np.int16)
    rr_sl = np.zeros(TOT, np.float32)
    val_sl = np.zeros(TOT, np.float32)
    idx_sl[gslot] = erel[sidx]
    rr_sl[gslot] = er[sidx].astype(np.float32)
    val_sl[gslot] = np.asarray(edge_val, np.float32)[sidx]
    sig = np.arange(TOT, dtype=np.int64) % GSLOT
    gof = np.arange(TOT, dtype=np.int64) // GSLOT
    padmask = (sig >= counts[gof]) & (sig < cnt128[gof] * 128)
    idx_sl[padmask] = 0
    gcnt_all = (cnt128 * 128).astype(np.int32)

    xTp = np.zeros((c.IN_C, c.TABROWS), BF16)
    xTp[:, :N] = np.asarray(x, np.float32).T.astype(BF16)

    per_core = []
    idx_c = idx_sl.reshape(c.CORES, WPC, Q * GSLOT)
    rr_c = rr_sl.reshape(c.CORES, -1, 128)
    val_c = val_sl.reshape(c.CORES, -1, 128)
    for k in range(c.CORES):
        a = idx_c[k].reshape(WPC, Q * GSLOT // 16, 16)
        w16 = np.transpose(a, (2, 0, 1)).reshape(16, WPC * Q * GSLOT // 16)
        idxw_k = np.ascontiguousarray(np.tile(w16, (8, 1)))
        rrT = rr_c[k].T  # [128, GCT]
        valT = val_c[k].T
        rrval_k = np.ascontiguousarray(np.stack([rrT, valT], axis=-1).reshape(128, -1))
        per_core.append(dict(idxw=idxw_k, rrval=rrval_k,
                             gcnt=np.ascontiguousarray(np.tile(gcnt_all.reshape(c.CORES, -1)[k:k + 1], (128, 1)))))
    meta = dict(core=core, win_of=win_of, slot_of=slot_of, xTp=xTp)
    return (per_core, meta), None


def make_in_maps(inputs, c: Cfg):
    res, cq_needed = preprocess(inputs["x"], np.asarray(inputs["edge_row"]),
                                np.asarray(inputs["edge_col"]), np.asarray(inputs["edge_val"]), c)
    if res is None:
        return None, cq_needed
    per_core, meta = res
    iota = np.broadcast_to(np.arange(128, dtype=np.float32), (128, 128)).astype(BF16)
    ident = np.eye(128, dtype=np.float32).astype(BF16)

    def v2(b):
        return np.ascontiguousarray(np.asarray(b, np.float32).reshape(2, 128).T)

    shared = dict(
        xT=meta["xTp"],
        wgcn=np.asarray(inputs["W_gcn"], np.float32),
        w1=np.asarray(inputs["W1"], np.float32),
        w2=np.asarray(inputs["W2"], np.float32),
        bgcn2=v2(inputs["b_gcn"]), g1v=v2(inputs["g1"]), be1v=v2(inputs["be1"]),
        g2v=v2(inputs["g2"]), be2v=v2(inputs["be2"]),
        iota=np.ascontiguousarray(iota), ident=np.ascontiguousarray(ident),
    )
    in_maps = [dict(shared, **pc) for pc in per_core]
    return (in_maps, meta), None


def unshard(results, meta, c: Cfg):
    core, win_of, slot_of = meta["core"], meta["win_of"], meta["slot_of"]
    out = np.empty((c.N, c.OUT_C), np.float32)
    rowpos = win_of.astype(np.int64) * 128 + slot_of
    for k in range(c.CORES):
        o = np.asarray(results[k]["outT"]).reshape(2, 128, c.RPC)
        o = np.transpose(o, (2, 0, 1)).reshape(c.RPC, c.OUT_C)
        nodes_k = np.flatnonzero(core == k)
        out[nodes_k] = o[rowpos[nodes_k]]
    return out


_NC_CACHE = {}


def get_nc(c: Cfg):
    key = (c.N, c.SEG, c.WPC, c.CQ, c.PH)
    if key not in _NC_CACHE:
        _NC_CACHE[key] = build(c)
    return _NC_CACHE[key]


def kernel(**inputs):
    c = Cfg()
    while True:
        res, cq_needed = make_in_maps(inputs, c)
        if res is not None:
            break
        c = Cfg(CQ=cq_needed)
    in_maps, meta = res
    nc = get_nc(c)
    r = run_bass_kernel_spmd(nc, in_maps, list(range(c.CORES)))
    return unshard(r.results, meta, c)



# revision 22
# speedup vs baseline: 5.8859x; 1.2133x over previous
"""Trainium2 Bass kernel for nn_Encoder (GCN layer + MLP/BatchNorm), 8 NeuronCores.

Strategy (per core, SPMD over 8 cores):
  Phase 1 (replicated): h = bf16(x @ W_gcn) written as a row-major DRAM table
    [Q*SEG, 256].  x arrives host-transposed AND host-cast (xT [512, Q*SEG]
    bf16, zero-padded) so the stationary operand needs no on-device transpose
    and the slab reads are half-width.  Slab loads ride sync-engine HWDGE in
    2 MiB groups of 4 slabs (49 DMAs instead of 196) and h-table stores ride
    scalar-engine HWDGE in 1 MiB groups of 16 row-groups (49 DMAs, single
    [TABROWS, 256] htab tensor), since this class of pod shows ~5us per-DMA-op
    overhead; the gpsimd Q7 SWDGE stays free for phase-2 gather descriptors.
  Phase 2 (sharded by destination row): edges are host-bucketed by
    (dest window of 128 rows, source quarter of the table) and sorted by
    source row within each bucket.  For each bucket a
    gpsimd dma_gather pulls the source rows (512B bf16 each, int16 indices
    relative to the quarter) into SBUF; a one-instruction DVE tensor_scalar
    builds S^T[j,r] = val_j * (rowrel_j == r) from a constant iota tile; the
    segment-sum becomes PE matmuls accumulating into a PSUM window.  The
    window is PE-transposed so downstream work is column-major (h_aggT).
  Phase 3: z1 = W1-matmuls (W1 stationary), BatchNorm stats are reduced
    locally and AllReduce'd across the 8 cores (payload [128,4] f32), then the
    window is recomputed and Relu+affine applied in one ACT op; same for W2 /
    BN2, final affine written f32 to outT [2,128,RPC].

Host side does only index/layout work: degree-balanced node->window
assignment (LPT), edge bucketing/sorting, padding, and the output unpermute.
"""

import heapq
import numpy as np
import ml_dtypes

import concourse.bacc as bacc
from concourse import bass, mybir
from concourse.bass_utils import run_bass_kernel_spmd
from concourse.library_config import mlp

BF16 = ml_dtypes.bfloat16
F32 = mybir.dt.float32
BF = mybir.dt.bfloat16
AF = mybir.ActivationFunctionType
OP = mybir.AluOpType


class Cfg:
    def __init__(self, N=100000, E=3200000, SEG=25088, WPC=98, CQ=10, PH=3):
        self.N, self.E, self.SEG, self.WPC, self.CQ = N, E, SEG, WPC, CQ
        self.PH = PH
        self.CORES = 8
        self.Q = 4
        self.IN_C, self.HID, self.OUT_C = 512, 256, 256
        self.EPS = 1e-5
        self.TABROWS = self.Q * SEG          # h-table rows (>= N, %128 == 0)
        assert self.TABROWS >= N and self.TABROWS % 128 == 0
        assert SEG <= 32767 and SEG % 512 == 0  # quarter tensors batch-aligned
        self.RG = self.TABROWS // 128        # phase-1 row groups
        self.GROWS = 2048                    # table rows per phase-1 slab group
        assert self.TABROWS % self.GROWS == 0
        self.SG = self.TABROWS // self.GROWS  # phase-1 slab groups (merged DMAs)
        self.RGPG = self.GROWS // 128        # row groups per slab group (16)
        self.RPC = WPC * 128                 # rows per core (padded)
        self.ROWS_REAL = N // self.CORES     # real rows per core
        assert self.ROWS_REAL <= self.RPC
        self.NCH = (self.RPC + 511) // 512   # phase-3 row chunks
        self.GSLOT = 128 * CQ                # slots per (window, quarter)
        self.NG = WPC * self.Q               # gather groups per core
        self.GCT = self.NG * CQ              # total chunk columns per core
        # rings
        self.R_IDX = min(8, WPC)
        self.R_RV = min(8, WPC)


def _ap(t, off, pattern):
    return bass.AP(t, off, pattern)


def build(c: Cfg):
    nc = bacc.Bacc("TRN2", debug=False)
    CQ, Q, WPC, SEG, HID = c.CQ, c.Q, c.WPC, c.SEG, c.HID

    xT = nc.declare_dram_parameter("xT", [c.IN_C, c.TABROWS], BF, isOutput=False)
    wgcn = nc.declare_dram_parameter("wgcn", [c.IN_C, HID], F32, isOutput=False)
    w1 = nc.declare_dram_parameter("w1", [HID, HID], F32, isOutput=False)
    w2 = nc.declare_dram_parameter("w2", [HID, c.OUT_C], F32, isOutput=False)
    bgcn2 = nc.declare_dram_parameter("bgcn2", [128, 2], F32, isOutput=False)
    g1v = nc.declare_dram_parameter("g1v", [128, 2], F32, isOutput=False)
    be1v = nc.declare_dram_parameter("be1v", [128, 2], F32, isOutput=False)
    g2v = nc.declare_dram_parameter("g2v", [128, 2], F32, isOutput=False)
    be2v = nc.declare_dram_parameter("be2v", [128, 2], F32, isOutput=False)
    iota_in = nc.declare_dram_parameter("iota", [128, 128], BF, isOutput=False)
    ident_in = nc.declare_dram_parameter("ident", [128, 128], BF, isOutput=False)
    idxw = nc.declare_dram_parameter("idxw", [128, WPC * 32 * CQ], mybir.dt.int16, isOutput=False)
    rrval = nc.declare_dram_parameter("rrval", [128, WPC * 8 * CQ], F32, isOutput=False)
    gcnt = nc.declare_dram_parameter("gcnt", [128, c.NG], mybir.dt.int32, isOutput=False)
    outT = nc.declare_dram_parameter("outT", [2, 128, c.RPC], F32, isOutput=True)

    htab = nc.dram_tensor("htab", [c.TABROWS, HID], BF)
    cc1i = nc.dram_tensor("cc1i", [128, 4], F32)
    cc1o = nc.dram_tensor("cc1o", [128, 4], F32, addr_space="Shared")
    cc2i = nc.dram_tensor("cc2i", [128, 4], F32)
    cc2o = nc.dram_tensor("cc2o", [128, 4], F32, addr_space="Shared")

    from contextlib import ExitStack
    st_ctx = ExitStack()
    T = lambda name, shape, dt: st_ctx.enter_context(nc.sbuf_tensor(name, shape, dt))
    P = lambda name, shape, dt=F32: st_ctx.enter_context(nc.psum_tensor(name, shape, dt))
    S = lambda name: st_ctx.enter_context(nc.semaphore(name))

    with st_ctx:
        xts = T("xts", [128, 2, 4, c.GROWS], BF)
        wg = T("wg", [128, 4, HID], BF)
        w1s = T("w1s", [128, 2, 2, 128], BF)
        w2s = T("w2s", [128, 2, 2, 128], BF)
        ht = T("ht", [128, 2, c.RGPG, HID], BF)
        ev1 = T("ev1", [128, 2, HID], BF)
        gr = T("gr", [128, 4, CQ, HID], BF)
        ss = T("ss", [128, 3, CQ, 128], BF)
        ixs = T("ixs", [128, c.R_IDX, 32 * CQ], mybir.dt.int16)
        rvs = T("rvs", [128, c.R_RV, 8 * CQ], F32)
        cnt = T("cnt", [128, c.NG], mybir.dt.int32)
        io_sb = T("io_sb", [128, 128], BF)
        id_sb = T("id_sb", [128, 128], BF)
        hat = T("hat", [128, 2, c.RPC], BF)
        h1 = T("h1", [128, 2, c.RPC], BF)
        bg = T("bg", [128, 2], F32)
        g1s = T("g1s", [128, 2], F32)
        be1s = T("be1s", [128, 2], F32)
        g2s = T("g2s", [128, 2], F32)
        be2s = T("be2s", [128, 2], F32)
        stt = T("stt", [128, 2, 2, c.NCH], F32)
        ccp = T("ccp", [128, 4], F32)
        gst = T("gst", [128, 8], F32)
        kdt = T("kdt", [128, 16], F32)
        kd1 = T("kd1", [128, 4], F32)
        kd2 = T("kd2", [128, 4], F32)
        ot = T("ot", [128, 2, 512], F32)

        pa = [P("pa0", [128, HID]), P("pa1", [128, HID])]
        pt = [P("pt0", [128, 2, 128], BF), P("pt1", [128, 2, 128], BF)]
        p3 = [P(f"p3{i}", [128, 512]) for i in range(4)]

        s_pre = S("s_pre"); s_ms = S("s_ms")
        s_slab = [S(f"s_slab{i}") for i in range(2)]
        s_p1ps = S("s_p1ps"); s_p1ev = S("s_p1ev")
        s_p1st = [S(f"s_p1st{i}") for i in range(2)]
        s_idx = [S(f"s_idx{i}") for i in range(c.R_IDX)]
        s_rv = [S(f"s_rv{i}") for i in range(c.R_RV)]
        s_g = [S(f"s_g{i}") for i in range(4)]
        s_s = S("s_s")
        s_pg = S("s_pg"); s_e1 = S("s_e1"); s_pt = S("s_pt"); s_e2 = S("s_e2")
        s_3ps = S("s_3ps"); s_3c = S("s_3c"); s_sq = S("s_sq"); s_h1 = S("s_h1"); s_oev = S("s_oev")
        s_ost = [S(f"s_ost{i}") for i in range(2)]
        s_stf = S("s_stf"); s_cio = S("s_cio"); s_cc = S("s_cc")
        s_kd = S("s_kd")

        N_PRE = 11 * 16
        # phase-3 chunk rows
        rows_t = [min(512, c.RPC - 512 * t) for t in range(c.NCH)]
        rreal_t = [max(0, min(rows_t[t], c.ROWS_REAL - 512 * t)) for t in range(c.NCH)]
        assert all(r > 0 for r in rreal_t)
        NT = 2 * c.NCH          # tiles per phase-3 pass
        # s_p1st targets when table fully stored (SG store groups, ring of 2)
        p1st_done = (16 * ((c.SG + 1) // 2), 16 * (c.SG // 2))

        with nc.Block() as block:

            @block.gpsimd
            def _(g: bass.BassGpSimd):
                g.load_library(mlp)
                # ---- preloads (11 DMAs) ----
                g.dma_start(wg[:, :, :], _ap(wgcn, 0, [[HID, 128], [128 * HID, 4], [1, HID]])).then_inc(s_pre, 16)
                g.dma_start(w1s[:, :, :, :], _ap(w1, 0, [[HID, 128], [128 * HID, 2], [128, 2], [1, 128]])).then_inc(s_pre, 16)
                g.dma_start(w2s[:, :, :, :], _ap(w2, 0, [[HID, 128], [128 * HID, 2], [128, 2], [1, 128]])).then_inc(s_pre, 16)
                g.dma_start(io_sb[:, :], iota_in[:, :]).then_inc(s_pre, 16)
                g.dma_start(id_sb[:, :], ident_in[:, :]).then_inc(s_pre, 16)
                g.dma_start(bg[:, :], bgcn2[:, :]).then_inc(s_pre, 16)
                g.dma_start(g1s[:, :], g1v[:, :]).then_inc(s_pre, 16)
                g.dma_start(be1s[:, :], be1v[:, :]).then_inc(s_pre, 16)
                g.dma_start(g2s[:, :], g2v[:, :]).then_inc(s_pre, 16)
                g.dma_start(be2s[:, :], be2v[:, :]).then_inc(s_pre, 16)
                g.dma_start(cnt[:, :], gcnt[:, :]).then_inc(s_pre, 16)
                # phase-1 slab loads / h-stores moved to sync / scalar (HWDGE)
                # so the Q7 SWDGE is free for phase-2 gather descriptor gen
                if c.PH < 2:
                    g.wait_ge(s_p1st[0], p1st_done[0])
                    g.wait_ge(s_p1st[1], p1st_done[1])
                    return
                # ---- phase 2: gathers ----
                g.wait_ge(s_p1st[0], p1st_done[0])
                g.wait_ge(s_p1st[1], p1st_done[1])
                g.wait_ge(s_ms, 4)
                with g.register("cntreg") as creg:
                    for w in range(WPC):
                        g.wait_ge(s_idx[w % c.R_IDX], 16 * (w // c.R_IDX + 1))
                        for q in range(Q):
                            gi = Q * w + q
                            if gi >= 4:
                                g.wait_ge(s_pg, gi - 3)
                            g.reg_load(creg, _ap(cnt, gi, [[c.NG, 1], [1, 1]]))
                            g.dma_gather(
                                gr[:, gi % 4, :, :],
                                _ap(htab, q * SEG * HID, [[HID, SEG], [1, HID]]),
                                ixs[:, w % c.R_IDX, q * 8 * CQ:(q + 1) * 8 * CQ],
                                c.GSLOT, creg, HID, single_packet=False,
                            ).then_inc(s_g[gi % 4], 16)
                if c.PH < 3:
                    return
                # ---- phase 3: stats AllReduce x2 ----
                g.wait_ge(s_stf, 1)
                g.dma_start(cc1i[:, :], ccp[:, :]).then_inc(s_cio, 16)
                g.wait_ge(s_cio, 16)
                g.collective_compute("AllReduce", OP.add, replica_groups=[list(range(c.CORES))],
                                     ins=[cc1i.ap().opt()], outs=[cc1o.ap().opt()]).then_inc(s_cc, 1)
                g.wait_ge(s_cc, 1)
                g.dma_start(gst[:, 0:4], cc1o[:, :]).then_inc(s_cio, 16)
                g.wait_ge(s_stf, 2)
                g.dma_start(cc2i[:, :], ccp[:, :]).then_inc(s_cio, 16)
                g.wait_ge(s_cio, 48)
                g.collective_compute("AllReduce", OP.add, replica_groups=[list(range(c.CORES))],
                                     ins=[cc2i.ap().opt()], outs=[cc2o.ap().opt()]).then_inc(s_cc, 1)
                g.wait_ge(s_cc, 2)
                g.dma_start(gst[:, 4:8], cc2o[:, :]).then_inc(s_cio, 16)
                g.wait_ge(s_cio, 64)

            @block.sync
            def _(sp):
                # ---- phase 1 xT slab-group loads (HWDGE, 2 MiB per DMA) ----
                for sg in range(c.SG):
                    if sg >= 2:
                        # PE must have consumed slab group sg-2 (16 rgs each)
                        sp.wait_ge(s_p1ps, c.RGPG * (sg - 1))
                    sp.dma_start(
                        xts[:, sg % 2, :, :],
                        _ap(xT, c.GROWS * sg, [[c.TABROWS, 128], [128 * c.TABROWS, 4], [1, c.GROWS]]),
                    ).then_inc(s_slab[sg % 2], 16)
                if c.PH < 2:
                    return
                # phase 2 idx + rrval window streams
                for w in range(WPC):
                    if w >= c.R_IDX:
                        wp = w - c.R_IDX
                        for q in range(Q):
                            sp.wait_ge(s_g[q], 16 * (wp + 1))
                    sp.dma_start(ixs[:, w % c.R_IDX, :], idxw[:, w * 32 * CQ:(w + 1) * 32 * CQ]).then_inc(s_idx[w % c.R_IDX], 16)
                    if w >= c.R_RV:
                        sp.wait_ge(s_s, CQ * Q * (w - c.R_RV + 1))
                    sp.dma_start(rvs[:, w % c.R_RV, :], rrval[:, w * 8 * CQ:(w + 1) * 8 * CQ]).then_inc(s_rv[w % c.R_RV], 16)
                if c.PH < 3:
                    for w in range(max(0, WPC - c.R_IDX), WPC):
                        sp.wait_ge(s_idx[w % c.R_IDX], 16 * (w // c.R_IDX + 1))
                        sp.wait_ge(s_rv[w % c.R_RV], 16 * (w // c.R_RV + 1))
                    return
                # phase 3 out stores
                for i in range(NT):
                    hf, t = divmod(i, c.NCH)
                    sp.wait_ge(s_oev, i + 1)
                    sp.dma_start(
                        _ap(outT, hf * 128 * c.RPC + t * 512, [[c.RPC, 128], [1, rows_t[t]]]),
                        ot[:, i % 2, 0:rows_t[t]],
                    ).then_inc(s_ost[i % 2], 16)
                sp.wait_ge(s_ost[0], 16 * ((NT + 1) // 2))
                sp.wait_ge(s_ost[1], 16 * (NT // 2))

            @block.tensor
            def _(pe: bass.BassTensorEngine):
                pe.wait_ge(s_pre, N_PRE)
                # ---- phase 1 matmuls ----
                for sg in range(c.SG):
                    pe.wait_ge(s_slab[sg % 2], 16 * (sg // 2 + 1))
                    for j in range(c.RGPG):
                        rg = c.RGPG * sg + j
                        if rg >= 2:
                            pe.wait_ge(s_p1ev, rg - 1)
                        for kc in range(4):
                            mm = pe.matmul(
                                pa[rg % 2][:, :],
                                xts[:, sg % 2, kc, 128 * j:128 * (j + 1)],
                                wg[:, kc, :],
                                start=(kc == 0), stop=(kc == 3),
                            )
                            if kc == 3:
                                mm.then_inc(s_p1ps, 1)
                if c.PH < 2:
                    return
                # ---- phase 2 scatter matmuls + window transposes ----
                for w in range(WPC):
                    if w >= 2:
                        pe.wait_ge(s_e1, w - 1)
                    for q in range(Q):
                        gi = Q * w + q
                        pe.wait_ge(s_g[gi % 4], 16 * (gi // 4 + 1))
                        pe.wait_ge(s_s, CQ * (gi + 1))
                        for ch in range(CQ):
                            mm = pe.matmul(
                                pa[w % 2][:, :],
                                ss[:, gi % 3, ch, :],
                                gr[:, gi % 4, ch, :],
                                start=(q == 0 and ch == 0), stop=(q == Q - 1 and ch == CQ - 1),
                            )
                            if ch == CQ - 1:
                                mm.then_inc(s_pg, 1)
                    if w >= 1:
                        v = w - 1
                        pe.wait_ge(s_e1, v + 1)
                        if v >= 2:
                            pe.wait_ge(s_e2, 2 * v - 2)
                        for i in range(2):
                            pe.matmul(pt[v % 2][:, i, :], ev1[:, v % 2, 128 * i:128 * (i + 1)],
                                      id_sb[:, :], is_transpose=True, start=True, stop=True).then_inc(s_pt, 1)
                v = WPC - 1
                pe.wait_ge(s_e1, v + 1)
                pe.wait_ge(s_e2, max(0, 2 * v - 2))
                for i in range(2):
                    pe.matmul(pt[v % 2][:, i, :], ev1[:, v % 2, 128 * i:128 * (i + 1)],
                              id_sb[:, :], is_transpose=True, start=True, stop=True).then_inc(s_pt, 1)
                if c.PH < 3:
                    return
                # ---- phase 3: 4 passes x (2 halves x NCH chunks) ----
                pe.wait_ge(s_e2, 2 * WPC)
                for i in range(4 * NT):
                    p, j = divmod(i, NT)
                    hf, t = divmod(j, c.NCH)
                    if i >= 4:
                        pp, jj = divmod(i - 4, NT)
                        if pp == 0:
                            pe.wait_ge(s_sq, jj + 1)
                        elif pp == 1:
                            pe.wait_ge(s_h1, jj + 1)
                        elif pp == 2:
                            pe.wait_ge(s_sq, NT + jj + 1)
                        else:
                            pe.wait_ge(s_oev, jj + 1)
                    if p == 2:
                        pe.wait_ge(s_h1, c.NCH + t + 1)
                    ws = w1s if p < 2 else w2s
                    src = hat if p < 2 else h1
                    rt = rows_t[t]
                    for ci in range(2):
                        mm = pe.matmul(
                            p3[i % 4][:, 0:rt],
                            ws[:, ci, hf, :],
                            src[:, ci, 512 * t:512 * t + rt],
                            start=(ci == 0), stop=(ci == 1),
                        )
                        if ci == 1:
                            mm.then_inc(s_3ps, 1)

            @block.vector
            def _(v: bass.BassVectorEngine):
                for sl4 in range(4):
                    v.memset(gr[:, sl4, :, :], 0).then_inc(s_ms, 1)
                v.wait_ge(s_pre, N_PRE)
                if c.PH < 2:
                    return
                # ---- phase 2 S-builds ----
                for w in range(WPC):
                    v.wait_ge(s_rv[w % c.R_RV], 16 * (w // c.R_RV + 1))
                    for q in range(Q):
                        gi = Q * w + q
                        if gi >= 3:
                            v.wait_ge(s_pg, gi - 2)
                        for ch in range(CQ):
                            v.tensor_scalar(
                                ss[:, gi % 3, ch, :], io_sb[:, :],
                                rvs[:, w % c.R_RV, 2 * (q * CQ + ch):2 * (q * CQ + ch) + 1],
                                rvs[:, w % c.R_RV, 2 * (q * CQ + ch) + 1:2 * (q * CQ + ch) + 2],
                                OP.is_equal, OP.mult,
                            ).then_inc(s_s, 1)
                if c.PH < 3:
                    return
                # ---- phase 3 ----
                for layer in range(2):
                    base = 0 if layer == 0 else 2 * NT
                    for j in range(NT):
                        hf, t = divmod(j, c.NCH)
                        v.wait_ge(s_3ps, base + j + 1)
                        rr = rreal_t[t]
                        psl = p3[(base + j) % 4]
                        v.tensor_reduce(stt[:, hf, 0, t:t + 1], psl[:, 0:rr],
                                        mybir.AxisListType.X, OP.add).then_inc(s_3c, 1)
                    v.wait_ge(s_sq, NT * (layer + 1))
                    v.drain()
                    v.tensor_reduce(ccp[:, 0:1], stt[:, 0, 0, :], mybir.AxisListType.X, OP.add)
                    v.tensor_reduce(ccp[:, 1:2], stt[:, 0, 1, :], mybir.AxisListType.X, OP.add)
                    v.tensor_reduce(ccp[:, 2:3], stt[:, 1, 0, :], mybir.AxisListType.X, OP.add)
                    v.tensor_reduce(ccp[:, 3:4], stt[:, 1, 1, :], mybir.AxisListType.X, OP.add)
                    v.drain().then_inc(s_stf, 1)
                    # finalize after AllReduce
                    v.wait_ge(s_cio, 32 + 32 * layer)
                    gof = 4 * layer
                    sums = _ap(gst, gof, [[8, 128], [2, 2]])
                    sqs = _ap(gst, gof + 1, [[8, 128], [2, 2]])
                    inv_n = 1.0 / c.N
                    v.tensor_scalar(kdt[:, 0:2], sums, inv_n, None, OP.mult)
                    v.tensor_scalar(kdt[:, 2:4], sqs, inv_n, None, OP.mult)
                    v.drain()
                    v.tensor_mul(kdt[:, 4:6], kdt[:, 0:2], kdt[:, 0:2])
                    v.drain()
                    v.tensor_sub(kdt[:, 6:8], kdt[:, 2:4], kdt[:, 4:6])
                    v.drain()
                    v.tensor_scalar(kdt[:, 6:8], kdt[:, 6:8], c.EPS, None, OP.add)
                    v.drain().then_inc(s_kd, 1)
                    v.wait_ge(s_kd, 2 + 3 * layer)
                    v.reciprocal(kdt[:, 10:12], kdt[:, 8:10])
                    v.drain()
                    kd = kd1 if layer == 0 else kd2
                    gv = g1s if layer == 0 else g2s
                    bev = be1s if layer == 0 else be2s
                    v.tensor_mul(kd[:, 0:2], gv[:, :], kdt[:, 10:12])
                    v.drain()
                    v.tensor_mul(kdt[:, 12:14], kdt[:, 0:2], kd[:, 0:2])
                    v.drain()
                    v.tensor_sub(kd[:, 2:4], bev[:, :], kdt[:, 12:14])
                    v.drain().then_inc(s_kd, 1)

            @block.scalar
            def _(a: bass.BassScalarEngine):
                a.wait_ge(s_pre, N_PRE)
                # ---- phase 1 psum evacuation (f32 -> bf16) + merged h-store (1 MiB HWDGE) ----
                for rg in range(c.RG):
                    a.wait_ge(s_p1ps, rg + 1)
                    b = rg // c.RGPG     # store group
                    if b >= 2 and rg % c.RGPG == 0:
                        a.wait_ge(s_p1st[b % 2], 16 * ((b - 2) // 2 + 1))
                    a.activation(ht[:, b % 2, rg % c.RGPG, :], pa[rg % 2][:, :], AF.Identity).then_inc(s_p1ev, 1)
                    if rg % c.RGPG == c.RGPG - 1:
                        a.dma_start(
                            _ap(htab, b * c.GROWS * HID, [[HID, 128], [128 * HID, c.RGPG], [1, HID]]),
                            ht[:, b % 2, :, :],
                        ).then_inc(s_p1st[b % 2], 16)
                if c.PH < 2:
                    return
                # ---- phase 2 evacuations ----
                for w in range(WPC):
                    a.wait_ge(s_pg, Q * (w + 1))
                    if w >= 2:
                        a.wait_ge(s_pt, 2 * w - 2)
                    a.activation(ev1[:, w % 2, :], pa[w % 2][:, :], AF.Identity).then_inc(s_e1, 1)
                    if w >= 1:
                        vv = w - 1
                        a.wait_ge(s_pt, 2 * (vv + 1))
                        for i in range(2):
                            a.activation(hat[:, i, 128 * vv:128 * (vv + 1)], pt[vv % 2][:, i, :],
                                         AF.Identity, bias=bg[:, i:i + 1]).then_inc(s_e2, 1)
                vv = WPC - 1
                a.wait_ge(s_pt, 2 * (vv + 1))
                for i in range(2):
                    a.activation(hat[:, i, 128 * vv:128 * (vv + 1)], pt[vv % 2][:, i, :],
                                 AF.Identity, bias=bg[:, i:i + 1]).then_inc(s_e2, 1)
                if c.PH < 3:
                    return
                # ---- phase 3 ----
                for layer in range(2):
                    sbase = 0 if layer == 0 else 2 * NT
                    for j in range(NT):
                        hf, t = divmod(j, c.NCH)
                        a.wait_ge(s_3ps, sbase + j + 1)
                        a.wait_ge(s_3c, NT * layer + j + 1)
                        rr = rreal_t[t]
                        psl = p3[(sbase + j) % 4]
                        a.activation(psl[:, 0:rr], psl[:, 0:rr], AF.Square,
                                     accum_out=stt[:, hf, 1, t:t + 1]).then_inc(s_sq, 1)
                    # sqrt step for k/d
                    a.wait_ge(s_kd, 1 + 3 * layer)
                    a.sqrt(kdt[:, 8:10], kdt[:, 6:8]).then_inc(s_kd, 1)
                    a.wait_ge(s_kd, 3 + 3 * layer)
                    kd = kd1 if layer == 0 else kd2
                    pbase = NT if layer == 0 else 3 * NT
                    for j in range(NT):
                        hf, t = divmod(j, c.NCH)
                        a.wait_ge(s_3ps, pbase + j + 1)
                        rt = rows_t[t]
                        psl = p3[(pbase + j) % 4]
                        if layer == 0:
                            a.activation(h1[:, hf, 512 * t:512 * t + rt], psl[:, 0:rt], AF.Relu,
                                         bias=kd[:, 2 + hf:3 + hf], scale=kd[:, hf:hf + 1]).then_inc(s_h1, 1)
                        else:
                            if j >= 2:
                                a.wait_ge(s_ost[j % 2], 16 * ((j - 2) // 2 + 1))
                            a.activation(ot[:, j % 2, 0:rt], psl[:, 0:rt], AF.Identity,
                                         bias=kd[:, 2 + hf:3 + hf], scale=kd[:, hf:hf + 1]).then_inc(s_oev, 1)

        nc.compile()
    return nc


# ---------------------------------------------------------------------------
# host-side preprocessing
# ---------------------------------------------------------------------------

def preprocess(x, edge_row, edge_col, edge_val, c: Cfg):
    N, E, WPC, Q, SEG = c.N, len(edge_row), c.WPC, c.Q, c.SEG
    deg = np.bincount(edge_row, minlength=N)
    order = np.argsort(-deg, kind="stable")
    rank = np.empty(N, np.int64)
    rank[order] = np.arange(N)
    core = (rank % c.CORES).astype(np.int32)

    win_of = np.empty(N, np.int32)
    slot_of = np.empty(N, np.int32)
    caps = np.full(WPC, 128, np.int32)
    tail = c.ROWS_REAL - 128 * (WPC - 1)
    caps[WPC - 1] = tail if tail > 0 else 128
    # capacity sanity: total capacity >= rows_real
    assert caps.sum() >= c.ROWS_REAL
    for k in range(c.CORES):
        nodes = order[k::c.CORES]
        heap = [(0, w) for w in range(WPC)]
        heapq.heapify(heap)
        fill = np.zeros(WPC, np.int32)
        for n in nodes:
            s, w = heapq.heappop(heap)
            win_of[n] = w
            slot_of[n] = fill[w]
            fill[w] += 1
            if fill[w] < caps[w]:
                heapq.heappush(heap, (s + int(deg[n]), w))
    ek = core[edge_row]
    ew = win_of[edge_row]
    er = slot_of[edge_row]
    eq = (edge_col // SEG).astype(np.int64)
    erel = (edge_col - eq * SEG).astype(np.int16)
    key = ((ek.astype(np.int64) * WPC + ew) * Q + eq)
    # secondary sort by source row: ascending gather addresses within a group
    sidx = np.lexsort((erel, key))
    key_s = key[sidx]
    ngroups = c.CORES * WPC * Q
    counts = np.bincount(key_s, minlength=ngroups)
    cnt128 = np.maximum((counts + 127) // 128, 1)
    cq_needed = int(cnt128.max())
    if cq_needed > c.CQ:
        return None, cq_needed  # caller rebuilds with larger CQ
    GSLOT = c.GSLOT
    starts = np.zeros(ngroups, np.int64)
    starts[1:] = np.cumsum(counts)[:-1]
    pos = np.arange(E) - starts[key_s]
    gslot = key_s * GSLOT + pos
    TOT = ngroups * GSLOT
    idx_sl = np.full(TOT, -1, np.int16)
    rr_sl = np.zeros(TOT, np.float32)
    val_sl = np.zeros(TOT, np.float32)
    idx_sl[gslot] = erel[sidx]
    rr_sl[gslot] = er[sidx].astype(np.float32)
    val_sl[gslot] = np.asarray(edge_val, np.float32)[sidx]
    sig = np.arange(TOT, dtype=np.int64) % GSLOT
    gof = np.arange(TOT, dtype=np.int64) // GSLOT
    padmask = (sig >= counts[gof]) & (sig < cnt128[gof] * 128)
    idx_sl[padmask] = 0
    gcnt_all = (cnt128 * 128).astype(np.int32)

    xTp = np.zeros((c.IN_C, c.TABROWS), BF16)
    xTp[:, :N] = np.asarray(x, np.float32).T.astype(BF16)

    per_core = []
    idx_c = idx_sl.reshape(c.CORES, WPC, Q * GSLOT)
    rr_c = rr_sl.reshape(c.CORES, -1, 128)
    val_c = val_sl.reshape(c.CORES, -1, 128)
    for k in range(c.CORES):
        a = idx_c[k].reshape(WPC, Q * GSLOT // 16, 16)
        w16 = np.transpose(a, (2, 0, 1)).reshape(16, WPC * Q * GSLOT // 16)
        idxw_k = np.ascontiguousarray(np.tile(w16, (8, 1)))
        rrT = rr_c[k].T  # [128, GCT]
        valT = val_c[k].T
        rrval_k = np.ascontiguousarray(np.stack([rrT, valT], axis=-1).reshape(128, -1))
        per_core.append(dict(idxw=idxw_k, rrval=rrval_k,
                             gcnt=np.ascontiguousarray(np.tile(gcnt_all.reshape(c.CORES, -1)[k:k + 1], (128, 1)))))
    meta = dict(core=core, win_of=win_of, slot_of=slot_of, xTp=xTp)
    return (per_core, meta), None


def make_in_maps(inputs, c: Cfg):
    res, cq_needed = preprocess(inputs["x"], np.asarray(inputs["edge_row"]),
                                np.asarray(inputs["edge_col"]), np.asarray(inputs["edge_val"]), c)
    if res is None:
        return None, cq_needed
    per_core, meta = res
    iota = np.broadcast_to(np.arange(128, dtype=np.float32), (128, 128)).astype(BF16)
    ident = np.eye(128, dtype=np.float32).astype(BF16)

    def v2(b):
        return np.ascontiguousarray(np.asarray(b, np.float32).reshape(2, 128).T)

    shared = dict(
        xT=meta["xTp"],
        wgcn=np.asarray(inputs["W_gcn"], np.float32),
        w1=np.asarray(inputs["W1"], np.float32),
        w2=np.asarray(inputs["W2"], np.float32),
        bgcn2=v2(inputs["b_gcn"]), g1v=v2(inputs["g1"]), be1v=v2(inputs["be1"]),
        g2v=v2(inputs["g2"]), be2v=v2(inputs["be2"]),
        iota=np.ascontiguousarray(iota), ident=np.ascontiguousarray(ident),
    )
    in_maps = [dict(shared, **pc) for pc in per_core]
    return (in_maps, meta), None


def unshard(results, meta, c: Cfg):
    core, win_of, slot_of = meta["core"], meta["win_of"], meta["slot_of"]
    out = np.empty((c.N, c.OUT_C), np.float32)
    rowpos = win_of.astype(np.int64) * 128 + slot_of
    for k in range(c.CORES):
        o = np.asarray(results[k]["outT"]).reshape(2, 128, c.RPC)
        o = np.transpose(o, (2, 0, 1)).reshape(c.RPC, c.OUT_C)
        nodes_k = np.flatnonzero(core == k)
        out[nodes_k] = o[rowpos[nodes_k]]
    return out


_NC_CACHE = {}


def get_nc(c: Cfg):
    key = (c.N, c.SEG, c.WPC, c.CQ, c.PH)
    if key not in _NC_CACHE:
        _NC_CACHE[key] = build(c)
    return _NC_CACHE[key]


def kernel(**inputs):
    c = Cfg()
    while True:
        res, cq_needed = make_in_maps(inputs, c)
        if res is not None:
            break
        c = Cfg(CQ=cq_needed)
    in_maps, meta = res
    nc = get_nc(c)
    r = run_bass_kernel_spmd(nc, in_maps, list(range(c.CORES)))
    return unshard(r.results, meta, c)



# revision 23
# speedup vs baseline: 6.2885x; 1.0684x over previous
"""Trainium2 Bass kernel for nn_Encoder (GCN layer + MLP/BatchNorm), 8 NeuronCores.

Strategy (per core, SPMD over 8 cores):
  Phase 1 (replicated): h = bf16(x @ W_gcn) written as a row-major DRAM table
    [Q*SEG, 256].  x arrives host-transposed AND host-cast (xT [512, Q*SEG]
    bf16, zero-padded) so the stationary operand needs no on-device transpose
    and the slab reads are half-width.  Slab loads ride sync-engine HWDGE in
    2 MiB groups of 4 slabs (49 DMAs instead of 196) and h-table stores ride
    scalar-engine HWDGE in 1 MiB groups of 16 row-groups (49 DMAs, single
    [TABROWS, 256] htab tensor), since this class of pod shows ~5us per-DMA-op
    overhead; the gpsimd Q7 SWDGE stays free for phase-2 gather descriptors.
  Phase 2 (sharded by destination row): edges are host-bucketed by
    (dest window of 128 rows, source quarter of the table) and sorted by
    source row within each bucket.  For each bucket a
    gpsimd dma_gather pulls the source rows (512B bf16 each, int16 indices
    relative to the quarter) into SBUF; a one-instruction DVE tensor_scalar
    builds S^T[j,r] = val_j * (rowrel_j == r) from a constant iota tile; the
    segment-sum becomes PE matmuls accumulating into a PSUM window.  The
    window is PE-transposed so downstream work is column-major (h_aggT).
  Phase 3: z1 = W1-matmuls (W1 stationary), BatchNorm stats are reduced
    locally and AllReduce'd across the 8 cores (payload [128,4] f32), then the
    window is recomputed and Relu+affine applied in one ACT op; same for W2 /
    BN2, final affine written f32 to outT [2,128,RPC].

Host side does only index/layout work: degree-balanced node->window
assignment (LPT), edge bucketing/sorting, padding, and the output unpermute.
"""

import heapq
import numpy as np
import ml_dtypes

import concourse.bacc as bacc
from concourse import bass, mybir
from concourse.bass_utils import run_bass_kernel_spmd
from concourse.library_config import mlp

BF16 = ml_dtypes.bfloat16
F32 = mybir.dt.float32
BF = mybir.dt.bfloat16
AF = mybir.ActivationFunctionType
OP = mybir.AluOpType


class Cfg:
    def __init__(self, N=100000, E=3200000, SEG=25088, WPC=98, CQ=10, PH=3):
        self.N, self.E, self.SEG, self.WPC, self.CQ = N, E, SEG, WPC, CQ
        self.PH = PH
        self.CORES = 8
        self.Q = 4
        self.IN_C, self.HID, self.OUT_C = 512, 256, 256
        self.EPS = 1e-5
        self.TABROWS = self.Q * SEG          # h-table rows (>= N, %128 == 0)
        assert self.TABROWS >= N and self.TABROWS % 128 == 0
        assert SEG <= 32767 and SEG % 512 == 0  # quarter tensors batch-aligned
        self.RG = self.TABROWS // 128        # phase-1 row groups
        self.GROWS = 2048                    # table rows per phase-1 slab group
        assert self.TABROWS % self.GROWS == 0
        self.SG = self.TABROWS // self.GROWS  # phase-1 slab groups (merged DMAs)
        self.RGPG = self.GROWS // 128        # row groups per slab group (16)
        self.RPC = WPC * 128                 # rows per core (padded)
        self.ROWS_REAL = N // self.CORES     # real rows per core
        assert self.ROWS_REAL <= self.RPC
        self.NCH = (self.RPC + 511) // 512   # phase-3 row chunks
        self.GSLOT = 128 * CQ                # slots per (window, quarter)
        self.NG = WPC * self.Q               # gather groups per core
        self.GCT = self.NG * CQ              # total chunk columns per core
        # rings: idx/rrval stream in groups of 4 windows, double-buffered
        self.WG = 4
        self.NWG = (WPC + self.WG - 1) // self.WG
        self.cumw = [min(self.WG * (g + 1), WPC) for g in range(self.NWG)]


def _ap(t, off, pattern):
    return bass.AP(t, off, pattern)


def build(c: Cfg):
    nc = bacc.Bacc("TRN2", debug=False)
    CQ, Q, WPC, SEG, HID = c.CQ, c.Q, c.WPC, c.SEG, c.HID

    xT = nc.declare_dram_parameter("xT", [c.IN_C, c.TABROWS], BF, isOutput=False)
    wgcn = nc.declare_dram_parameter("wgcn", [c.IN_C, HID], F32, isOutput=False)
    w1 = nc.declare_dram_parameter("w1", [HID, HID], F32, isOutput=False)
    w2 = nc.declare_dram_parameter("w2", [HID, c.OUT_C], F32, isOutput=False)
    bgcn2 = nc.declare_dram_parameter("bgcn2", [128, 2], F32, isOutput=False)
    g1v = nc.declare_dram_parameter("g1v", [128, 2], F32, isOutput=False)
    be1v = nc.declare_dram_parameter("be1v", [128, 2], F32, isOutput=False)
    g2v = nc.declare_dram_parameter("g2v", [128, 2], F32, isOutput=False)
    be2v = nc.declare_dram_parameter("be2v", [128, 2], F32, isOutput=False)
    iota_in = nc.declare_dram_parameter("iota", [128, 128], BF, isOutput=False)
    ident_in = nc.declare_dram_parameter("ident", [128, 128], BF, isOutput=False)
    idxw = nc.declare_dram_parameter("idxw", [128, WPC * 32 * CQ], mybir.dt.int16, isOutput=False)
    rrval = nc.declare_dram_parameter("rrval", [128, WPC * 8 * CQ], F32, isOutput=False)
    gcnt = nc.declare_dram_parameter("gcnt", [128, c.NG], mybir.dt.int32, isOutput=False)
    outT = nc.declare_dram_parameter("outT", [2, 128, c.RPC], F32, isOutput=True)

    htab = nc.dram_tensor("htab", [c.TABROWS, HID], BF)
    cc1i = nc.dram_tensor("cc1i", [128, 4], F32)
    cc1o = nc.dram_tensor("cc1o", [128, 4], F32, addr_space="Shared")
    cc2i = nc.dram_tensor("cc2i", [128, 4], F32)
    cc2o = nc.dram_tensor("cc2o", [128, 4], F32, addr_space="Shared")

    from contextlib import ExitStack
    st_ctx = ExitStack()
    T = lambda name, shape, dt: st_ctx.enter_context(nc.sbuf_tensor(name, shape, dt))
    P = lambda name, shape, dt=F32: st_ctx.enter_context(nc.psum_tensor(name, shape, dt))
    S = lambda name: st_ctx.enter_context(nc.semaphore(name))

    with st_ctx:
        xts = T("xts", [128, 2, 4, c.GROWS], BF)
        wg = T("wg", [128, 4, HID], BF)
        w1s = T("w1s", [128, 2, 2, 128], BF)
        w2s = T("w2s", [128, 2, 2, 128], BF)
        ht = T("ht", [128, 2, c.RGPG, HID], BF)
        ev1 = T("ev1", [128, 2, HID], BF)
        gr = T("gr", [128, 4, CQ, HID], BF)
        ss = T("ss", [128, 3, CQ, 128], BF)
        ixs = T("ixs", [128, 2, c.WG * 32 * CQ], mybir.dt.int16)
        rvs = T("rvs", [128, 2, c.WG * 8 * CQ], F32)
        cnt = T("cnt", [128, c.NG], mybir.dt.int32)
        io_sb = T("io_sb", [128, 128], BF)
        id_sb = T("id_sb", [128, 128], BF)
        hat = T("hat", [128, 2, c.RPC], BF)
        h1 = T("h1", [128, 2, c.RPC], BF)
        bg = T("bg", [128, 2], F32)
        g1s = T("g1s", [128, 2], F32)
        be1s = T("be1s", [128, 2], F32)
        g2s = T("g2s", [128, 2], F32)
        be2s = T("be2s", [128, 2], F32)
        stt = T("stt", [128, 2, 2, c.NCH], F32)
        ccp = T("ccp", [128, 4], F32)
        gst = T("gst", [128, 8], F32)
        kdt = T("kdt", [128, 16], F32)
        kd1 = T("kd1", [128, 4], F32)
        kd2 = T("kd2", [128, 4], F32)
        ot = T("ot", [128, 2, 512], F32)

        pa = [P("pa0", [128, HID]), P("pa1", [128, HID])]
        pt = [P("pt0", [128, 2, 128], BF), P("pt1", [128, 2, 128], BF)]
        p3 = [P(f"p3{i}", [128, 512]) for i in range(4)]

        s_pre = S("s_pre"); s_ms = S("s_ms")
        s_slab = [S(f"s_slab{i}") for i in range(2)]
        s_p1ps = S("s_p1ps"); s_p1ev = S("s_p1ev")
        s_p1st = [S(f"s_p1st{i}") for i in range(2)]
        s_idx = [S(f"s_idx{i}") for i in range(2)]
        s_rv = [S(f"s_rv{i}") for i in range(2)]
        s_g = [S(f"s_g{i}") for i in range(4)]
        s_s = S("s_s")
        s_pg = S("s_pg"); s_e1 = S("s_e1"); s_pt = S("s_pt"); s_e2 = S("s_e2")
        s_3ps = S("s_3ps"); s_3c = S("s_3c"); s_sq = S("s_sq"); s_h1 = S("s_h1"); s_oev = S("s_oev")
        s_ost = [S(f"s_ost{i}") for i in range(2)]
        s_stf = S("s_stf"); s_cio = S("s_cio"); s_cc = S("s_cc")
        s_kd = S("s_kd")

        N_PRE = 11 * 16
        # phase-3 chunk rows
        rows_t = [min(512, c.RPC - 512 * t) for t in range(c.NCH)]
        rreal_t = [max(0, min(rows_t[t], c.ROWS_REAL - 512 * t)) for t in range(c.NCH)]
        assert all(r > 0 for r in rreal_t)
        NT = 2 * c.NCH          # tiles per phase-3 pass
        # s_p1st targets when table fully stored (SG store groups, ring of 2)
        p1st_done = (16 * ((c.SG + 1) // 2), 16 * (c.SG // 2))

        with nc.Block() as block:

            @block.gpsimd
            def _(g: bass.BassGpSimd):
                g.load_library(mlp)
                # ---- preloads (11 DMAs) ----
                g.dma_start(wg[:, :, :], _ap(wgcn, 0, [[HID, 128], [128 * HID, 4], [1, HID]])).then_inc(s_pre, 16)
                g.dma_start(w1s[:, :, :, :], _ap(w1, 0, [[HID, 128], [128 * HID, 2], [128, 2], [1, 128]])).then_inc(s_pre, 16)
                g.dma_start(w2s[:, :, :, :], _ap(w2, 0, [[HID, 128], [128 * HID, 2], [128, 2], [1, 128]])).then_inc(s_pre, 16)
                g.dma_start(io_sb[:, :], iota_in[:, :]).then_inc(s_pre, 16)
                g.dma_start(id_sb[:, :], ident_in[:, :]).then_inc(s_pre, 16)
                g.dma_start(bg[:, :], bgcn2[:, :]).then_inc(s_pre, 16)
                g.dma_start(g1s[:, :], g1v[:, :]).then_inc(s_pre, 16)
                g.dma_start(be1s[:, :], be1v[:, :]).then_inc(s_pre, 16)
                g.dma_start(g2s[:, :], g2v[:, :]).then_inc(s_pre, 16)
                g.dma_start(be2s[:, :], be2v[:, :]).then_inc(s_pre, 16)
                g.dma_start(cnt[:, :], gcnt[:, :]).then_inc(s_pre, 16)
                # phase-1 slab loads / h-stores moved to sync / scalar (HWDGE)
                # so the Q7 SWDGE is free for phase-2 gather descriptor gen
                if c.PH < 2:
                    g.wait_ge(s_p1st[0], p1st_done[0])
                    g.wait_ge(s_p1st[1], p1st_done[1])
                    return
                # ---- phase 2: gathers ----
                g.wait_ge(s_p1st[0], p1st_done[0])
                g.wait_ge(s_p1st[1], p1st_done[1])
                g.wait_ge(s_ms, 4)
                with g.register("cntreg") as creg:
                    for w in range(WPC):
                        gw = w // c.WG
                        g.wait_ge(s_idx[gw % 2], 16 * (gw // 2 + 1))
                        for q in range(Q):
                            gi = Q * w + q
                            if gi >= 4:
                                g.wait_ge(s_pg, gi - 3)
                            g.reg_load(creg, _ap(cnt, gi, [[c.NG, 1], [1, 1]]))
                            g.dma_gather(
                                gr[:, gi % 4, :, :],
                                _ap(htab, q * SEG * HID, [[HID, SEG], [1, HID]]),
                                ixs[:, (w // c.WG) % 2, ((w % c.WG) * Q + q) * 8 * CQ:((w % c.WG) * Q + q + 1) * 8 * CQ],
                                c.GSLOT, creg, HID, single_packet=False,
                            ).then_inc(s_g[gi % 4], 16)
                if c.PH < 3:
                    return
                # ---- phase 3: stats AllReduce x2 ----
                g.wait_ge(s_stf, 1)
                g.dma_start(cc1i[:, :], ccp[:, :]).then_inc(s_cio, 16)
                g.wait_ge(s_cio, 16)
                g.collective_compute("AllReduce", OP.add, replica_groups=[list(range(c.CORES))],
                                     ins=[cc1i.ap().opt()], outs=[cc1o.ap().opt()]).then_inc(s_cc, 1)
                g.wait_ge(s_cc, 1)
                g.dma_start(gst[:, 0:4], cc1o[:, :]).then_inc(s_cio, 16)
                g.wait_ge(s_stf, 2)
                g.dma_start(cc2i[:, :], ccp[:, :]).then_inc(s_cio, 16)
                g.wait_ge(s_cio, 48)
                g.collective_compute("AllReduce", OP.add, replica_groups=[list(range(c.CORES))],
                                     ins=[cc2i.ap().opt()], outs=[cc2o.ap().opt()]).then_inc(s_cc, 1)
                g.wait_ge(s_cc, 2)
                g.dma_start(gst[:, 4:8], cc2o[:, :]).then_inc(s_cio, 16)
                g.wait_ge(s_cio, 64)

            @block.sync
            def _(sp):
                # ---- phase 1 xT slab-group loads (HWDGE, 2 MiB per DMA) ----
                for sg in range(c.SG):
                    if sg >= 2:
                        # PE must have consumed slab group sg-2 (16 rgs each)
                        sp.wait_ge(s_p1ps, c.RGPG * (sg - 1))
                    sp.dma_start(
                        xts[:, sg % 2, :, :],
                        _ap(xT, c.GROWS * sg, [[c.TABROWS, 128], [128 * c.TABROWS, 4], [1, c.GROWS]]),
                    ).then_inc(s_slab[sg % 2], 16)
                if c.PH < 2:
                    return
                # phase 2 idx + rrval streams, 4-window group DMAs
                for gw in range(c.NWG):
                    w0 = c.WG * gw
                    nw = c.cumw[gw] - w0
                    if gw >= 2:
                        done_w = c.cumw[gw - 2]
                        for q in range(Q):
                            sp.wait_ge(s_g[q], 16 * done_w)
                        sp.wait_ge(s_s, CQ * Q * done_w)
                    sp.dma_start(ixs[:, gw % 2, 0:nw * 32 * CQ],
                                 idxw[:, w0 * 32 * CQ:(w0 + nw) * 32 * CQ]).then_inc(s_idx[gw % 2], 16)
                    sp.dma_start(rvs[:, gw % 2, 0:nw * 8 * CQ],
                                 rrval[:, w0 * 8 * CQ:(w0 + nw) * 8 * CQ]).then_inc(s_rv[gw % 2], 16)
                if c.PH < 3:
                    for gw in (c.NWG - 2, c.NWG - 1):
                        sp.wait_ge(s_idx[gw % 2], 16 * (gw // 2 + 1))
                        sp.wait_ge(s_rv[gw % 2], 16 * (gw // 2 + 1))
                    return
                # phase 3 out stores
                for i in range(NT):
                    hf, t = divmod(i, c.NCH)
                    sp.wait_ge(s_oev, i + 1)
                    sp.dma_start(
                        _ap(outT, hf * 128 * c.RPC + t * 512, [[c.RPC, 128], [1, rows_t[t]]]),
                        ot[:, i % 2, 0:rows_t[t]],
                    ).then_inc(s_ost[i % 2], 16)
                sp.wait_ge(s_ost[0], 16 * ((NT + 1) // 2))
                sp.wait_ge(s_ost[1], 16 * (NT // 2))

            @block.tensor
            def _(pe: bass.BassTensorEngine):
                pe.wait_ge(s_pre, N_PRE)
                # ---- phase 1 matmuls ----
                for sg in range(c.SG):
                    pe.wait_ge(s_slab[sg % 2], 16 * (sg // 2 + 1))
                    for j in range(c.RGPG):
                        rg = c.RGPG * sg + j
                        if rg >= 2:
                            pe.wait_ge(s_p1ev, rg - 1)
                        for kc in range(4):
                            mm = pe.matmul(
                                pa[rg % 2][:, :],
                                xts[:, sg % 2, kc, 128 * j:128 * (j + 1)],
                                wg[:, kc, :],
                                start=(kc == 0), stop=(kc == 3),
                            )
                            if kc == 3:
                                mm.then_inc(s_p1ps, 1)
                if c.PH < 2:
                    return
                # ---- phase 2 scatter matmuls + window transposes ----
                for w in range(WPC):
                    if w >= 2:
                        pe.wait_ge(s_e1, w - 1)
                    for q in range(Q):
                        gi = Q * w + q
                        pe.wait_ge(s_g[gi % 4], 16 * (gi // 4 + 1))
                        pe.wait_ge(s_s, CQ * (gi + 1))
                        for ch in range(CQ):
                            mm = pe.matmul(
                                pa[w % 2][:, :],
                                ss[:, gi % 3, ch, :],
                                gr[:, gi % 4, ch, :],
                                start=(q == 0 and ch == 0), stop=(q == Q - 1 and ch == CQ - 1),
                            )
                            if ch == CQ - 1:
                                mm.then_inc(s_pg, 1)
                    if w >= 1:
                        v = w - 1
                        pe.wait_ge(s_e1, v + 1)
                        if v >= 2:
                            pe.wait_ge(s_e2, 2 * v - 2)
                        for i in range(2):
                            pe.matmul(pt[v % 2][:, i, :], ev1[:, v % 2, 128 * i:128 * (i + 1)],
                                      id_sb[:, :], is_transpose=True, start=True, stop=True).then_inc(s_pt, 1)
                v = WPC - 1
                pe.wait_ge(s_e1, v + 1)
                pe.wait_ge(s_e2, max(0, 2 * v - 2))
                for i in range(2):
                    pe.matmul(pt[v % 2][:, i, :], ev1[:, v % 2, 128 * i:128 * (i + 1)],
                              id_sb[:, :], is_transpose=True, start=True, stop=True).then_inc(s_pt, 1)
                if c.PH < 3:
                    return
                # ---- phase 3: 4 passes x (2 halves x NCH chunks) ----
                pe.wait_ge(s_e2, 2 * WPC)
                for i in range(4 * NT):
                    p, j = divmod(i, NT)
                    hf, t = divmod(j, c.NCH)
                    if i >= 4:
                        pp, jj = divmod(i - 4, NT)
                        if pp == 0:
                            pe.wait_ge(s_sq, jj + 1)
                        elif pp == 1:
                            pe.wait_ge(s_h1, jj + 1)
                        elif pp == 2:
                            pe.wait_ge(s_sq, NT + jj + 1)
                        else:
                            pe.wait_ge(s_oev, jj + 1)
                    if p == 2:
                        pe.wait_ge(s_h1, c.NCH + t + 1)
                    ws = w1s if p < 2 else w2s
                    src = hat if p < 2 else h1
                    rt = rows_t[t]
                    for ci in range(2):
                        mm = pe.matmul(
                            p3[i % 4][:, 0:rt],
                            ws[:, ci, hf, :],
                            src[:, ci, 512 * t:512 * t + rt],
                            start=(ci == 0), stop=(ci == 1),
                        )
                        if ci == 1:
                            mm.then_inc(s_3ps, 1)

            @block.vector
            def _(v: bass.BassVectorEngine):
                for sl4 in range(4):
                    v.memset(gr[:, sl4, :, :], 0).then_inc(s_ms, 1)
                v.wait_ge(s_pre, N_PRE)
                if c.PH < 2:
                    return
                # ---- phase 2 S-builds ----
                for w in range(WPC):
                    gw = w // c.WG
                    v.wait_ge(s_rv[gw % 2], 16 * (gw // 2 + 1))
                    for q in range(Q):
                        gi = Q * w + q
                        if gi >= 3:
                            v.wait_ge(s_pg, gi - 2)
                        for ch in range(CQ):
                            v.tensor_scalar(
                                ss[:, gi % 3, ch, :], io_sb[:, :],
                                rvs[:, (w // c.WG) % 2, 2 * ((w % c.WG) * Q * CQ + q * CQ + ch):2 * ((w % c.WG) * Q * CQ + q * CQ + ch) + 1],
                                rvs[:, (w // c.WG) % 2, 2 * ((w % c.WG) * Q * CQ + q * CQ + ch) + 1:2 * ((w % c.WG) * Q * CQ + q * CQ + ch) + 2],
                                OP.is_equal, OP.mult,
                            ).then_inc(s_s, 1)
                if c.PH < 3:
                    return
                # ---- phase 3 ----
                for layer in range(2):
                    base = 0 if layer == 0 else 2 * NT
                    for j in range(NT):
                        hf, t = divmod(j, c.NCH)
                        v.wait_ge(s_3ps, base + j + 1)
                        rr = rreal_t[t]
                        psl = p3[(base + j) % 4]
                        v.tensor_reduce(stt[:, hf, 0, t:t + 1], psl[:, 0:rr],
                                        mybir.AxisListType.X, OP.add).then_inc(s_3c, 1)
                    v.wait_ge(s_sq, NT * (layer + 1))
                    v.drain()
                    v.tensor_reduce(ccp[:, 0:1], stt[:, 0, 0, :], mybir.AxisListType.X, OP.add)
                    v.tensor_reduce(ccp[:, 1:2], stt[:, 0, 1, :], mybir.AxisListType.X, OP.add)
                    v.tensor_reduce(ccp[:, 2:3], stt[:, 1, 0, :], mybir.AxisListType.X, OP.add)
                    v.tensor_reduce(ccp[:, 3:4], stt[:, 1, 1, :], mybir.AxisListType.X, OP.add)
                    v.drain().then_inc(s_stf, 1)
                    # finalize after AllReduce
                    v.wait_ge(s_cio, 32 + 32 * layer)
                    gof = 4 * layer
                    sums = _ap(gst, gof, [[8, 128], [2, 2]])
                    sqs = _ap(gst, gof + 1, [[8, 128], [2, 2]])
                    inv_n = 1.0 / c.N
                    v.tensor_scalar(kdt[:, 0:2], sums, inv_n, None, OP.mult)
                    v.tensor_scalar(kdt[:, 2:4], sqs, inv_n, None, OP.mult)
                    v.drain()
                    v.tensor_mul(kdt[:, 4:6], kdt[:, 0:2], kdt[:, 0:2])
                    v.drain()
                    v.tensor_sub(kdt[:, 6:8], kdt[:, 2:4], kdt[:, 4:6])
                    v.drain()
                    v.tensor_scalar(kdt[:, 6:8], kdt[:, 6:8], c.EPS, None, OP.add)
                    v.drain().then_inc(s_kd, 1)
                    v.wait_ge(s_kd, 2 + 3 * layer)
                    v.reciprocal(kdt[:, 10:12], kdt[:, 8:10])
                    v.drain()
                    kd = kd1 if layer == 0 else kd2
                    gv = g1s if layer == 0 else g2s
                    bev = be1s if layer == 0 else be2s
                    v.tensor_mul(kd[:, 0:2], gv[:, :], kdt[:, 10:12])
                    v.drain()
                    v.tensor_mul(kdt[:, 12:14], kdt[:, 0:2], kd[:, 0:2])
                    v.drain()
                    v.tensor_sub(kd[:, 2:4], bev[:, :], kdt[:, 12:14])
                    v.drain().then_inc(s_kd, 1)

            @block.scalar
            def _(a: bass.BassScalarEngine):
                a.wait_ge(s_pre, N_PRE)
                # ---- phase 1 psum evacuation (f32 -> bf16) + merged h-store (1 MiB HWDGE) ----
                for rg in range(c.RG):
                    a.wait_ge(s_p1ps, rg + 1)
                    b = rg // c.RGPG     # store group
                    if b >= 2 and rg % c.RGPG == 0:
                        a.wait_ge(s_p1st[b % 2], 16 * ((b - 2) // 2 + 1))
                    a.activation(ht[:, b % 2, rg % c.RGPG, :], pa[rg % 2][:, :], AF.Identity).then_inc(s_p1ev, 1)
                    if rg % c.RGPG == c.RGPG - 1:
                        a.dma_start(
                            _ap(htab, b * c.GROWS * HID, [[HID, 128], [128 * HID, c.RGPG], [1, HID]]),
                            ht[:, b % 2, :, :],
                        ).then_inc(s_p1st[b % 2], 16)
                if c.PH < 2:
                    return
                # ---- phase 2 evacuations ----
                for w in range(WPC):
                    a.wait_ge(s_pg, Q * (w + 1))
                    if w >= 2:
                        a.wait_ge(s_pt, 2 * w - 2)
                    a.activation(ev1[:, w % 2, :], pa[w % 2][:, :], AF.Identity).then_inc(s_e1, 1)
                    if w >= 1:
                        vv = w - 1
                        a.wait_ge(s_pt, 2 * (vv + 1))
                        for i in range(2):
                            a.activation(hat[:, i, 128 * vv:128 * (vv + 1)], pt[vv % 2][:, i, :],
                                         AF.Identity, bias=bg[:, i:i + 1]).then_inc(s_e2, 1)
                vv = WPC - 1
                a.wait_ge(s_pt, 2 * (vv + 1))
                for i in range(2):
                    a.activation(hat[:, i, 128 * vv:128 * (vv + 1)], pt[vv % 2][:, i, :],
                                 AF.Identity, bias=bg[:, i:i + 1]).then_inc(s_e2, 1)
                if c.PH < 3:
                    return
                # ---- phase 3 ----
                for layer in range(2):
                    sbase = 0 if layer == 0 else 2 * NT
                    for j in range(NT):
                        hf, t = divmod(j, c.NCH)
                        a.wait_ge(s_3ps, sbase + j + 1)
                        a.wait_ge(s_3c, NT * layer + j + 1)
                        rr = rreal_t[t]
                        psl = p3[(sbase + j) % 4]
                        a.activation(psl[:, 0:rr], psl[:, 0:rr], AF.Square,
                                     accum_out=stt[:, hf, 1, t:t + 1]).then_inc(s_sq, 1)
                    # sqrt step for k/d
                    a.wait_ge(s_kd, 1 + 3 * layer)
                    a.sqrt(kdt[:, 8:10], kdt[:, 6:8]).then_inc(s_kd, 1)
                    a.wait_ge(s_kd, 3 + 3 * layer)
                    kd = kd1 if layer == 0 else kd2
                    pbase = NT if layer == 0 else 3 * NT
                    for j in range(NT):
                        hf, t = divmod(j, c.NCH)
                        a.wait_ge(s_3ps, pbase + j + 1)
                        rt = rows_t[t]
                        psl = p3[(pbase + j) % 4]
                        if layer == 0:
                            a.activation(h1[:, hf, 512 * t:512 * t + rt], psl[:, 0:rt], AF.Relu,
                                         bias=kd[:, 2 + hf:3 + hf], scale=kd[:, hf:hf + 1]).then_inc(s_h1, 1)
                        else:
                            if j >= 2:
                                a.wait_ge(s_ost[j % 2], 16 * ((j - 2) // 2 + 1))
                            a.activation(ot[:, j % 2, 0:rt], psl[:, 0:rt], AF.Identity,
                                         bias=kd[:, 2 + hf:3 + hf], scale=kd[:, hf:hf + 1]).then_inc(s_oev, 1)

        nc.compile()
    return nc


# ---------------------------------------------------------------------------
# host-side preprocessing
# ---------------------------------------------------------------------------

def preprocess(x, edge_row, edge_col, edge_val, c: Cfg):
    N, E, WPC, Q, SEG = c.N, len(edge_row), c.WPC, c.Q, c.SEG
    deg = np.bincount(edge_row, minlength=N)
    order = np.argsort(-deg, kind="stable")
    rank = np.empty(N, np.int64)
    rank[order] = np.arange(N)
    core = (rank % c.CORES).astype(np.int32)

    win_of = np.empty(N, np.int32)
    slot_of = np.empty(N, np.int32)
    caps = np.full(WPC, 128, np.int32)
    tail = c.ROWS_REAL - 128 * (WPC - 1)
    caps[WPC - 1] = tail if tail > 0 else 128
    # capacity sanity: total capacity >= rows_real
    assert caps.sum() >= c.ROWS_REAL
    for k in range(c.CORES):
        nodes = order[k::c.CORES]
        heap = [(0, w) for w in range(WPC)]
        heapq.heapify(heap)
        fill = np.zeros(WPC, np.int32)
        for n in nodes:
            s, w = heapq.heappop(heap)
            win_of[n] = w
            slot_of[n] = fill[w]
            fill[w] += 1
            if fill[w] < caps[w]:
                heapq.heappush(heap, (s + int(deg[n]), w))
    ek = core[edge_row]
    ew = win_of[edge_row]
    er = slot_of[edge_row]
    eq = (edge_col // SEG).astype(np.int64)
    erel = (edge_col - eq * SEG).astype(np.int16)
    key = ((ek.astype(np.int64) * WPC + ew) * Q + eq)
    # secondary sort by source row: ascending gather addresses within a group
    sidx = np.lexsort((erel, key))
    key_s = key[sidx]
    ngroups = c.CORES * WPC * Q
    counts = np.bincount(key_s, minlength=ngroups)
    cnt128 = np.maximum((counts + 127) // 128, 1)
    cq_needed = int(cnt128.max())
    if cq_needed > c.CQ:
        return None, cq_needed  # caller rebuilds with larger CQ
    GSLOT = c.GSLOT
    starts = np.zeros(ngroups, np.int64)
    starts[1:] = np.cumsum(counts)[:-1]
    pos = np.arange(E) - starts[key_s]
    gslot = key_s * GSLOT + pos
    TOT = ngroups * GSLOT
    idx_sl = np.full(TOT, -1, np.int16)
    rr_sl = np.zeros(TOT, np.float32)
    val_sl = np.zeros(TOT, np.float32)
    idx_sl[gslot] = erel[sidx]
    rr_sl[gslot] = er[sidx].astype(np.float32)
    val_sl[gslot] = np.asarray(edge_val, np.float32)[sidx]
    sig = np.arange(TOT, dtype=np.int64) % GSLOT
    gof = np.arange(TOT, dtype=np.int64) // GSLOT
    padmask = (sig >= counts[gof]) & (sig < cnt128[gof] * 128)
    idx_sl[padmask] = 0
    gcnt_all = (cnt128 * 128).astype(np.int32)

    xTp = np.zeros((c.IN_C, c.TABROWS), BF16)
    xTp[:, :N] = np.asarray(x, np.float32).T.astype(BF16)

    per_core = []
    idx_c = idx_sl.reshape(c.CORES, WPC, Q * GSLOT)
    rr_c = rr_sl.reshape(c.CORES, -1, 128)
    val_c = val_sl.reshape(c.CORES, -1, 128)
    for k in range(c.CORES):
        a = idx_c[k].reshape(WPC, Q * GSLOT // 16, 16)
        w16 = np.transpose(a, (2, 0, 1)).reshape(16, WPC * Q * GSLOT // 16)
        idxw_k = np.ascontiguousarray(np.tile(w16, (8, 1)))
        rrT = rr_c[k].T  # [128, GCT]
        valT = val_c[k].T
        rrval_k = np.ascontiguousarray(np.stack([rrT, valT], axis=-1).reshape(128, -1))
        per_core.append(dict(idxw=idxw_k, rrval=rrval_k,
                             gcnt=np.ascontiguousarray(np.tile(gcnt_all.reshape(c.CORES, -1)[k:k + 1], (128, 1)))))
    meta = dict(core=core, win_of=win_of, slot_of=slot_of, xTp=xTp)
    return (per_core, meta), None


def make_in_maps(inputs, c: Cfg):
    res, cq_needed = preprocess(inputs["x"], np.asarray(inputs["edge_row"]),
                                np.asarray(inputs["edge_col"]), np.asarray(inputs["edge_val"]), c)
    if res is None:
        return None, cq_needed
    per_core, meta = res
    iota = np.broadcast_to(np.arange(128, dtype=np.float32), (128, 128)).astype(BF16)
    ident = np.eye(128, dtype=np.float32).astype(BF16)

    def v2(b):
        return np.ascontiguousarray(np.asarray(b, np.float32).reshape(2, 128).T)

    shared = dict(
        xT=meta["xTp"],
        wgcn=np.asarray(inputs["W_gcn"], np.float32),
        w1=np.asarray(inputs["W1"], np.float32),
        w2=np.asarray(inputs["W2"], np.float32),
        bgcn2=v2(inputs["b_gcn"]), g1v=v2(inputs["g1"]), be1v=v2(inputs["be1"]),
        g2v=v2(inputs["g2"]), be2v=v2(inputs["be2"]),
        iota=np.ascontiguousarray(iota), ident=np.ascontiguousarray(ident),
    )
    in_maps = [dict(shared, **pc) for pc in per_core]
    return (in_maps, meta), None


def unshard(results, meta, c: Cfg):
    core, win_of, slot_of = meta["core"], meta["win_of"], meta["slot_of"]
    out = np.empty((c.N, c.OUT_C), np.float32)
    rowpos = win_of.astype(np.int64) * 128 + slot_of
    for k in range(c.CORES):
        o = np.asarray(results[k]["outT"]).reshape(2, 128, c.RPC)
        o = np.transpose(o, (2, 0, 1)).reshape(c.RPC, c.OUT_C)
        nodes_k = np.flatnonzero(core == k)
        out[nodes_k] = o[rowpos[nodes_k]]
    return out


_NC_CACHE = {}


def get_nc(c: Cfg):
    key = (c.N, c.SEG, c.WPC, c.CQ, c.PH)
    if key not in _NC_CACHE:
        _NC_CACHE[key] = build(c)
    return _NC_CACHE[key]


def kernel(**inputs):
    c = Cfg()
    while True:
        res, cq_needed = make_in_maps(inputs, c)
        if res is not None:
            break
        c = Cfg(CQ=cq_needed)
    in_maps, meta = res
    nc = get_nc(c)
    r = run_bass_kernel_spmd(nc, in_maps, list(range(c.CORES)))
    return unshard(r.results, meta, c)

